# revision 20
# baseline (speedup 1.0000x reference)
"""AttentionLSTM Trainium2 kernel — 8-core data-parallel.

Model (per batch row b): two independent single-direction LSTMs over T=43
steps of x[:, :, t] (H=300 features), hidden states summed, then a
conv-softmax attention over time, tanh, fc(300->80), softmax.

Device mapping per core (512 batch rows):
  - z^T[1200, 512] per (direction, step) via PE matmuls: merged contraction
    K=5 k-tiles of 128 (h rows 0..299 first, then bias + x tail, then x),
    M gate-aligned tiles {128,128,44}, fp16 MM inputs, fp32 PSUM accum.
  - h k-tiles are parity double-buffered: step t reads kt[t%2], writes h_t
    into kt[(t+1)%2], so every matmul of a step sees the full h_{t-1}
    (exact LSTM semantics, no Gauss-Seidel staleness).
  - the two directions' 44-row gate tails are PACKED into one PSUM pair
    (d0 at partitions 0:44, d1 at 64:108): ACT/DVE cost scales with
    free-dim columns only, so one elementwise chain covers both tails —
    the ACT engine (the bottleneck at ~96% busy) saves ~2.5us/step.
  - group order d0j0, d0j1, TAIL, d1j0, d1j1: the tail chain (whose h
    rows feed every mix pass next step) completes mid-step, removing the
    per-step PE stall the tail-at-end ordering caused.
  - gates: one fused sigmoid over an [*,1024] i|f PSUM pair, tanh(g) and
    sigmoid(o) over a shared g|o PSUM pair; gate/cell elementwise state in
    fp16 for 2x DVE throughput; c stays in SBUF; fp8 h copies for the DR
    matmuls run on GPSIMD (idle) instead of DVE.
  - attention accumulated online: e_t = sigmoid(a)/(1-sigmoid(a)) = exp(a)
    (avoids exp table loads mid-loop; 1-sigmoid on DVE, off the ACT
    engine); e_t broadcast across partitions with a rank-1 PE matmul
    (ones x e) into PSUM; r += on GPSIMD; one packed [128,1536] tanh for
    the three hsum segments; the softmax denominator rides along in
    r[2][44] via the mix tiles' bias rows (hs[44] = 2 every step).
  - tail: hStar = tanh(r/s), logits = fc(hStar) via PE (batch on PSUM
    partitions), softmax over the 80-class free dim.
"""

import os
import sys

sys.path.insert(0, "/opt/trn_rl_repo")

from contextlib import ExitStack

import numpy as np

import concourse.bass as bass
import concourse.tile as tile
from concourse import mybir
from concourse.bass_utils import run_bass_kernel_spmd  # noqa: F401  (spmd path kept available)

f32 = mybir.dt.float32
AF = mybir.ActivationFunctionType
AX = mybir.AxisListType

_BIRFIX_DONE = False


def _split_multiwaits(bir_json):
    """This walrus build allows one sync-wait per engine instruction; Tile
    attaches one per producer proc. Hoist extras onto standalone
    EventSemaphore instructions inserted just before, same engine queue."""
    import json
    j = json.loads(bir_json.decode() if isinstance(bir_json, bytes) else bir_json)
    for fn in j.get("functions", []):
        for blk in fn.get("blocks", []):
            out = []
            for ins in blk.get("instructions", []):
                si = ins.get("sync_info")
                ow = si.get("on_wait") if si else None
                if ow and len(ow) > 1:
                    for i, w in enumerate(ow[:-1]):
                        out.append({
                            "debug": ins.get("debug", 0),
                            "engine": ins["engine"],
                            "ins": [], "outs": [],
                            "name": f"{ins['name']}_xw{i}",
                            "opcode": "EventSemaphore",
                            "sync_info": {"on_update": [], "on_wait": [w]},
                        })
                    si["on_wait"] = [ow[-1]]
                out.append(ins)
            blk["instructions"] = out
    return json.dumps(j).encode()


def _install_birfix():
    global _BIRFIX_DONE
    if _BIRFIX_DONE:
        return
    from concourse import bass2jax
    orig = bass2jax.compile_bir_kernel

    def patched(bir_json, tmpdir, neff_name="file.neff"):
        return orig(_split_multiwaits(bir_json), tmpdir, neff_name)

    bass2jax.compile_bir_kernel = patched
    _BIRFIX_DONE = True


class _Runner:
    """Compile once; keep the sharded jitted executable + device inputs."""

    def __init__(self, nc, n_cores):
        import jax
        from jax.sharding import Mesh, PartitionSpec
        from jax.experimental.shard_map import shard_map
        from concourse import bass2jax as b2j

        b2j.install_neuronx_cc_hook()
        _install_birfix()
        self.jax = jax
        self.nc = nc
        self.n_cores = n_cores
        part_name = nc.partition_id_tensor.name if nc.partition_id_tensor else None
        in_names, out_names, out_avals, zero_outs = [], [], [], []
        for alloc in nc.m.functions[0].allocations:
            if not isinstance(alloc, mybir.MemoryLocationSet):
                continue
            name = alloc.memorylocations[0].name
            if alloc.kind == "ExternalInput":
                if name != part_name:
                    in_names.append(name)
            elif alloc.kind == "ExternalOutput":
                out_names.append(name)
                shape = tuple(alloc.tensor_shape)
                dtype = mybir.dt.np(alloc.dtype)
                out_avals.append(jax.core.ShapedArray(shape, dtype))
                zero_outs.append(np.zeros(shape, dtype))
        self.in_names = list(in_names)
        self.out_names = out_names
        self.out_avals = out_avals
        self.zero_outs = zero_outs
        n_params = len(in_names)
        n_outs = len(out_avals)
        all_names = in_names + out_names
        if part_name is not None:
            all_names = all_names + [part_name]
        donate = tuple(range(n_params, n_params + n_outs))

        def _body(*args):
            operands = list(args)
            if part_name is not None:
                operands.append(b2j.partition_id_tensor())
            outs = b2j._bass_exec_p.bind(
                *operands,
                out_avals=tuple(out_avals),
                in_names=tuple(all_names),
                out_names=tuple(out_names),
                lowering_input_output_aliases=(),
                sim_require_finite=True,
                sim_require_nnan=True,
                nc=nc,
            )
            return tuple(outs)

        devices = jax.devices()[:n_cores]
        self.mesh = Mesh(np.asarray(devices), ("core",))
        in_specs = (PartitionSpec("core"),) * (n_params + n_outs)
        out_specs = (PartitionSpec("core"),) * n_outs
        self.sharded = jax.jit(
            shard_map(_body, mesh=self.mesh, in_specs=in_specs,
                      out_specs=out_specs, check_rep=False),
            donate_argnums=donate, keep_unused=True)
        self.sharding = jax.sharding.NamedSharding(
            self.mesh, PartitionSpec("core"))

    def put_inputs(self, in_maps):
        jax = self.jax
        concat = [np.concatenate([np.asarray(m[n]) for m in in_maps], axis=0)
                  for n in self.in_names]
        return [jax.device_put(a, self.sharding) for a in concat]

    def call(self, dev_in):
        zeros = [np.zeros((self.n_cores * z.shape[0], *z.shape[1:]), z.dtype)
                 for z in self.zero_outs]
        outs = self.sharded(*dev_in, *zeros)
        self.jax.block_until_ready(outs)
        return outs

    def run(self, in_maps):
        dev_in = self.put_inputs(in_maps)
        outs = self.call(dev_in)
        n = self.n_cores
        return [
            {name: np.asarray(outs[i]).reshape(n, *self.out_avals[i].shape)[c]
             for i, name in enumerate(self.out_names)}
            for c in range(n)
        ]

    def bench(self, in_maps, iters=5):
        import time
        dev_in = self.put_inputs(in_maps)
        self.call(dev_in)  # warm
        times = []
        for _ in range(iters):
            t0 = time.perf_counter()
            self.call(dev_in)
            times.append(time.perf_counter() - t0)
        return times


B, H, T, NCLS = 4096, 300, 43, 80
NCORES = 8
BS = B // NCORES          # 512 batch rows per core
NK = 5                    # k-tiles: [h0:128 | h128:256 | h256:300+bias+xtail | x0:128 | x128:256]
BIASROW = 44              # partition of the bias (constant-1) row in k-tile 2
XTAIL = 64                # x rows 256..300 live at parts 64..108 of k-tile 2
MT = [(0, 128), (128, 128), (256, 44)]    # (moff, msz) per gate, output base partition 0
GOFF = [0, 300, 600, 900]                 # torch gate order i,f,g,o
NPAR = 2                                  # h k-tile phases (3-phase measured no better)
WDRW = 4 * 304                            # fp8 DR weights: 16B-aligned gate pitch

MM_DT_NAME = os.environ.get("LSTM_MM_DT", "float16")
# fp8e4m3 DoubleRow for the x[0:256] contraction: one 256-row matmul pass
# replaces two fp16 128-row passes (20% fewer gate matmul passes; emulated
# end-to-end rel err 8.2e-3 vs the 2e-2 gate).
USE_DR = os.environ.get("LSTM_X_FP8_DR", "1") == "1"

_CACHE = {}


def _build(mdt_name, repeat=0, variant="full"):
    # variant: "full" | "no_attn" (skip attention accumulation) |
    # "no_dve" (also skip the c/h elementwise chain) | "mm_only"
    # (matmuls + DMAs only) | "mm_nodma" (matmuls, static rhs) |
    # "mm_n256" (matmuls at N=256).  Non-"full" variants are timing probes.
    do_attn = variant == "full"
    do_dve = variant in ("full", "no_attn", "no_rec")
    do_act = variant not in ("mm_only", "mm_nodma", "mm_n256")
    do_xdma = variant != "mm_nodma"
    do_rec = variant != "no_rec"
    ncols = 256 if variant == "mm_n256" else 512
    mdt = getattr(mybir.dt, mdt_name)
    f8 = mybir.dt.float8e4
    DRMODE = mybir.MatmulPerfMode.DoubleRow
    from concourse.alu_op_type import AluOpType
    nc = bass.Bass(target_bir_lowering=False)

    xt_d = nc.declare_dram_parameter("xt", [T, 3, 128, BS], mdt, isOutput=False)
    if USE_DR:
        xdr_d = nc.declare_dram_parameter("xdr", [T, 128, 2, BS], f8,
                                          isOutput=False)
        wdr_d = nc.declare_dram_parameter("wdr", [2, 128, 2, WDRW], f8,
                                          isOutput=False)
        wdrh_d = nc.declare_dram_parameter("wdrh", [2, 128, 2, WDRW], f8,
                                           isOutput=False)
        # combined x gate-tail weights for the packed tail: per gate,
        # cols 0:44 = d0 tail rows, 64:108 = d1 tail rows (112 pitch)
        wdrxt_d = nc.declare_dram_parameter("wdrxt", [128, 2, 448], f8,
                                            isOutput=False)
    wc_d = nc.declare_dram_parameter("wc", [2, NK, 128, 1200], mdt, isOutput=False)
    conv_d = nc.declare_dram_parameter("convp", [128, 3], mdt, isOutput=False)
    fcw_d = nc.declare_dram_parameter("fcw", [128, 3 * NCLS], mdt, isOutput=False)
    fcb_d = nc.declare_dram_parameter("fcb", [1, NCLS], mdt, isOutput=False)
    ones_d = nc.declare_dram_parameter("onesrow", [1, BS], mdt, isOutput=False)
    sel44_d = nc.declare_dram_parameter("sel44", [128, 1], f32, isOutput=False)
    out_d = nc.declare_dram_parameter("out", [BS, NCLS], f32, isOutput=True)

    with tile.TileContext(nc) as tc, ExitStack() as ctx:
        P = lambda name, bufs, **kw: ctx.enter_context(
            tc.tile_pool(name=name, bufs=bufs, **kw))
        wpool = P("w", 1)
        xpool = P("x", 3)
        # One shared pool for all gate PSUM tiles: 3 x [128,1024] f32 =
        # 6 banks.  With separate zif(bufs=2)/zgo(bufs=1) pools the g|o
        # matmuls of each group waited on the previous group's o-act drain
        # with ~0 margin -> ~1us PE stall per group (~260us/forward).
        zp = P("z", 3, space="PSUM")
        # Attention score [1,512] and broadcast [128,512] share one slot
        # tag (strictly sequential within a step); 2 bufs = 2 banks.
        atp = P("at", 2, space="PSUM")
        sifp = P("sif", 3)
        sop = P("so", 3)
        gcp = P("gc", 1)
        p1p = P("p1", 3)
        tcp = P("tc", 3)
        hp = P("h", 1)
        hsp = P("hs", 2)
        thp = P("th", 2)
        rp = P("r", 1)
        smp = P("sm", 2)
        tmpp = P("tmp", 2)
        fin = P("fin", 2)

        # ---- weights / constants ----
        wc_sb = {}
        nk_sb = 3 if USE_DR else NK
        for d in range(2):
            for k in range(nk_sb):
                wt = wpool.tile([128, 1200], mdt, tag=f"wc_{d}_{k}")
                nc.sync.dma_start(out=wt, in_=wc_d.ap()[d, k])
                wc_sb[(d, k)] = wt
        wdr_sb = {}
        wdrh_sb = {}
        wdrxt_sb = None
        if USE_DR:
            for d in range(2):
                wt = wpool.tile([128, 2, WDRW], f8, tag=f"wdr_{d}")
                nc.sync.dma_start(out=wt, in_=wdr_d.ap()[d])
                wdr_sb[d] = wt
                wth = wpool.tile([128, 2, WDRW], f8, tag=f"wdrh_{d}")
                nc.sync.dma_start(out=wth, in_=wdrh_d.ap()[d])
                wdrh_sb[d] = wth
            wdrxt_sb = wpool.tile([128, 2, 448], f8, tag="wdrxt")
            nc.sync.dma_start(out=wdrxt_sb, in_=wdrxt_d.ap())
        conv_sb = wpool.tile([128, 3], mdt, tag="conv")
        nc.sync.dma_start(out=conv_sb, in_=conv_d.ap())
        fcw_sb = wpool.tile([128, 3 * NCLS], mdt, tag="fcw")
        nc.sync.dma_start(out=fcw_sb, in_=fcw_d.ap())
        fcb_sb = wpool.tile([1, NCLS], mdt, tag="fcb")
        nc.sync.dma_start(out=fcb_sb, in_=fcb_d.ap())
        ones_sb = wpool.tile([1, 128], mdt, tag="ones")
        nc.vector.memset(ones_sb, 1.0)
        sel44_sb = wpool.tile([128, 1], f32, tag="sel44")
        nc.sync.dma_start(out=sel44_sb, in_=sel44_d.ap())

        # ---- persistent state ----
        # h k-tiles, parity double-buffered: step t reads kt[t%2][d],
        # writes h_t into kt[(t+1)%2][d].
        kt = {}
        kt_dr = {}
        for par in range(NPAR):
            for d in range(2):
                kt[(par, d)] = []
                for j in range(3):
                    t_ = hp.tile([128, BS], mdt, tag=f"kt_{par}_{d}_{j}")
                    nc.vector.memset(t_, 0.0)
                    kt[(par, d)].append(t_)
                nc.sync.dma_start(out=kt[(par, d)][2][BIASROW:BIASROW + 1],
                                  in_=ones_d.ap())
                if USE_DR:
                    td = hp.tile([128, 2, BS], f8, tag=f"ktdr_{par}_{d}")
                    nc.vector.memset(td, 0.0)
                    kt_dr[(par, d)] = td
        # gate/cell state [tanh_g | c]: per (d, j) for the full 128-row
        # groups; ONE shared tile for the packed tails (d0 at partitions
        # 0:44, d1 at 64:108 — matmul col-group alignment needs base 64).
        gc = {}
        for d in range(2):
            for j in range(2):
                g = gcp.tile([128, 1024], mdt, tag=f"gc_{d}_{j}")
                nc.vector.memset(g, 0.0)
                gc[(d, j)] = g
        gc_t = gcp.tile([128, 1024], mdt, tag="gc_t")
        nc.vector.memset(gc_t, 0.0)
        r = []
        for j in range(3):
            rt = rp.tile([128, BS], f32, tag=f"r_{j}")
            nc.vector.memset(rt, 0.0)
            r.append(rt)
        # ssum is accumulated FOR FREE in r[2][44]: the mix k-tiles' bias
        # rows make hs[44] = 2 every step, so r[2][44] = 2 * sum_t e_t.
        # Zero the junk partitions of the two rotating hs buffers once so
        # the packed 1536-col tanh never sees NaN bit patterns.
        # hs rows 45:128 of the j2 segment are junk fed through the packed
        # tanh; nothing downstream reads those partitions of th (score and
        # accum slice [0:45]), so no zeroing is needed.

        def w_slice(d, k, col0, msz):
            return wc_sb[(d, k)][:, col0:col0 + msz]

        def attn_tanh(hs):
            # hs: [128, 1536] packed hsum (j0|j1|j2-tail) from the PREVIOUS
            # step.  One activation covers all three segments.
            th = thp.tile([128, 3 * BS], mdt, tag="th")
            nc.scalar.activation(out=th, in_=hs, func=AF.Tanh)
            return th

        def attn_score(th):
            a_ps = atp.tile([1, BS], f32, tag="at")
            for k in range(3):
                pmax = 45 if k == 2 else 128
                nc.tensor.matmul(a_ps, lhsT=conv_sb[0:pmax, k:k + 1],
                                 rhs=th[0:pmax, k * BS:k * BS + BS],
                                 start=(k == 0), stop=(k == 2))
            sg = smp.tile([1, BS], f32, tag="sg")
            nc.scalar.activation(out=sg, in_=a_ps, func=AF.Sigmoid)
            om = smp.tile([1, BS], f32, tag="om")
            nc.vector.tensor_scalar(out=om, in0=sg, scalar1=-1.0, scalar2=1.0,
                                    op0=AluOpType.mult, op1=AluOpType.add)
            nc.vector.reciprocal(out=om, in_=om)
            e16 = smp.tile([1, BS], mdt, tag="e16")
            nc.vector.tensor_mul(out=e16, in0=sg, in1=om)   # e = exp(a)
            return e16

        def attn_accum(hs, e16):
            eb_ps = atp.tile([128, BS], f32, tag="at")
            nc.tensor.matmul(eb_ps, lhsT=ones_sb, rhs=e16, start=True, stop=True)
            for j in range(3):
                pmax = 45 if j == 2 else 128
                tmp = tmpp.tile([128, BS], f32, tag=f"tmp{j}")
                nc.vector.tensor_mul(out=tmp[0:pmax],
                                     in0=hs[0:pmax, j * BS:j * BS + BS],
                                     in1=eb_ps[0:pmax])
                nc.gpsimd.tensor_add(out=r[j][0:pmax], in0=r[j][0:pmax],
                                     in1=tmp[0:pmax])

        def attn_tail(hs):
            attn_accum(hs, attn_score(attn_tanh(hs)))

        loop_cm = tc.For_i(0, repeat, 1) if repeat else None
        if loop_cm is not None:
            loop_cm.__enter__()

        pending_hs = None
        if not do_xdma:
            if USE_DR:
                xdr0 = xpool.tile([128, 2, BS], f8, tag="xdr")
                nc.vector.memset(xdr0, 0.0)
            else:
                xa0 = xpool.tile([128, BS], mdt, tag="xa")
                nc.vector.memset(xa0, 0.0)
                xb0 = xpool.tile([128, BS], mdt, tag="xb")
                nc.vector.memset(xb0, 0.0)

        # gate gi -> (z-tile index, col offset): i|f in zif, g|o in zgo
        GATE_COL = ((0, 0), (0, 512), (1, 0), (1, 512))

        # ---- time loop ----
        # Per-step group order: d0j0, d0j1, TAIL(packed, both dirs), d1j0,
        # d1j1.  The packed tail runs mid-step so its ACT/DVE chain (which
        # produces the h-tail rows every full group's mix pass needs next
        # step) completes ~2 groups before the step ends — the baseline's
        # tail-at-end ordering stalled PE ~1.4us at every step boundary.
        for t in range(T):
            par, nxt = t % NPAR, (t + 1) % NPAR
            xa = xb = xdr = None
            if do_xdma:
                if USE_DR:
                    xdr = xpool.tile([128, 2, BS], f8, tag="xdr")
                    nc.sync.dma_start(out=xdr, in_=xdr_d.ap()[t])
                else:
                    xa = xpool.tile([128, BS], mdt, tag="xa")
                    nc.sync.dma_start(out=xa, in_=xt_d.ap()[t, 0])
                    xb = xpool.tile([128, BS], mdt, tag="xb")
                    nc.sync.dma_start(out=xb, in_=xt_d.ap()[t, 1])
                for d in range(2):
                    nc.sync.dma_start(out=kt[(par, d)][2][XTAIL:XTAIL + 44],
                                      in_=xt_d.ap()[t, 2][XTAIL:XTAIL + 44])
            elif USE_DR:
                xdr = xdr0
            else:
                xa, xb = xa0, xb0
            # previous step's attention tanh: emitted first — its input has
            # been ready since last step, so ACT starts immediately while
            # PE fills the first gate group.
            pend_th = attn_tanh(pending_hs) if (do_attn and pending_hs) else None
            pend_e16 = None

            hs = (hsp.tile([128, 3 * BS], mdt, tag="hs", name="hs")
                  if do_attn else None)

            def gate_matmuls(d, zif, zgo, moff, msz, plo):
                """Passes accumulating z for (direction d, row-tile at
                moff..moff+msz), output partitions plo..plo+msz."""
                sl = slice(plo, plo + msz)
                for gi in range(4):
                    ti, c0 = GATE_COL[gi]
                    zdst = (zif, zgo)[ti][sl, c0:c0 + ncols]
                    col0 = GOFF[gi] + moff
                    if USE_DR and plo == 0:
                        dc0 = 304 * gi + moff   # 16B-aligned gate starts
                        nc.tensor.matmul(
                            zdst, lhsT=wdr_sb[d][:, :, dc0:dc0 + msz],
                            rhs=xdr[:, :, 0:ncols],
                            start=True, stop=False, perf_mode=DRMODE)
                        nc.tensor.matmul(
                            zdst, lhsT=wdrh_sb[d][:, :, dc0:dc0 + msz],
                            rhs=kt_dr[(par, d)][:, :, 0:ncols],
                            start=False, stop=False, perf_mode=DRMODE)
                        nc.tensor.matmul(
                            zdst, lhsT=w_slice(d, 2, col0, msz),
                            rhs=kt[(par, d)][2][:, 0:ncols],
                            start=False, stop=True)
                    elif USE_DR:
                        # DoubleRow can't target a nonzero PSUM col-group:
                        # contract the same fp8 tiles half-by-half with
                        # plain passes (fp8 streams at bf16 rate)
                        dc0 = 304 * gi + moff
                        for s in range(2):
                            nc.tensor.matmul(
                                zdst, lhsT=wdr_sb[d][:, s, dc0:dc0 + msz],
                                rhs=xdr[:, s, 0:ncols],
                                start=(s == 0), stop=False)
                        for s in range(2):
                            nc.tensor.matmul(
                                zdst, lhsT=wdrh_sb[d][:, s, dc0:dc0 + msz],
                                rhs=kt_dr[(par, d)][:, s, 0:ncols],
                                start=False, stop=False)
                        nc.tensor.matmul(
                            zdst, lhsT=w_slice(d, 2, col0, msz),
                            rhs=kt[(par, d)][2][:, 0:ncols],
                            start=False, stop=True)
                    else:
                        rhsk = [(0, kt[(par, d)][0]), (1, kt[(par, d)][1]),
                                (2, kt[(par, d)][2]), (3, xa), (4, xb)]
                        for ki, (wk, rtile) in enumerate(rhsk):
                            nc.tensor.matmul(
                                zdst, lhsT=w_slice(d, wk, col0, msz),
                                rhs=rtile[:, 0:ncols],
                                start=(ki == 0), stop=(ki == len(rhsk) - 1))

            def full_group(d, j):
                moff, msz = MT[j]
                sl = slice(0, msz)
                zif = zp.tile([128, 1024], f32, tag="z")
                zgo = zp.tile([128, 1024], f32, tag="z")
                gate_matmuls(d, zif, zgo, moff, msz, 0)
                if not do_act:
                    return
                sif = sifp.tile([128, 1024], mdt, tag="sif")
                nc.scalar.activation(out=sif[sl], in_=zif[sl], func=AF.Sigmoid)
                gcj = gc[(d, j)]
                nc.scalar.activation(out=gcj[sl, 0:512], in_=zgo[sl, 0:512],
                                     func=AF.Tanh)
                so = sop.tile([128, BS], mdt, tag="so")
                nc.scalar.activation(out=so[sl], in_=zgo[sl, 512:1024],
                                     func=AF.Sigmoid)
                if not do_dve:
                    return
                # c_new = sig_f * c + sig_i * tanh_g ; h = sig_o * tanh(c)
                p1 = p1p.tile([128, 1024], mdt, tag="p1")
                nc.vector.tensor_mul(out=p1[sl], in0=sif[sl], in1=gcj[sl])
                nc.vector.tensor_add(out=gcj[sl, 512:1024],
                                     in0=p1[sl, 0:512], in1=p1[sl, 512:1024])
                tcj = tcp.tile([128, BS], mdt, tag="tc")
                nc.scalar.activation(out=tcj[sl], in_=gcj[sl, 512:1024],
                                     func=AF.Tanh)
                # h_t lands directly in the next step's rhs k-tile
                if do_rec:
                    hdst = kt[(nxt, d)][j]
                else:  # timing probe: same traffic, no recurrence dep
                    hdst = tcp.tile([128, BS], mdt, tag="hscr")
                nc.vector.tensor_mul(out=hdst[sl], in0=so[sl], in1=tcj[sl])
                if USE_DR and do_rec:
                    # fp8 copy feeds next step's DR h matmul (GPSIMD — it
                    # has slack; DVE is the second-busiest engine); the
                    # fp16 original stays for the attention hsum path.
                    nc.gpsimd.tensor_copy(
                        out=kt_dr[(nxt, d)][:, j, :], in_=hdst)
                if d == 1 and do_attn:
                    nc.vector.tensor_add(out=hs[:, j * BS:j * BS + BS],
                                         in0=kt[(nxt, 0)][j],
                                         in1=kt[(nxt, 1)][j])

            def tail_group():
                """Both directions' 44-row gate tails in one PSUM pair:
                d0 at partitions 0:44, d1 at 64:108 (col-group aligned).
                One ACT/DVE chain covers both directions — activation and
                vector cost scale with free-dim columns only, so packing
                partitions halves the tails' elementwise cost."""
                zif = zp.tile([128, 1024], f32, tag="z")
                zgo = zp.tile([128, 1024], f32, tag="z")
                if USE_DR:
                    # 6 passes per gate, one accumulation group per bank:
                    # x for BOTH dirs in one DoubleRow pass (xdr is shared;
                    # the combined lhsT has d0 tail cols at 0:44, d1 at
                    # 64:108 — DR is legal at col-group 0), then h per dir
                    # (d0 as DR at base 0; d1 as two plain fp8 half passes
                    # since DR can't target col-group 64), then the two
                    # fp16 mix passes.
                    for gi in range(4):
                        ti, c0 = GATE_COL[gi]
                        zt = (zif, zgo)[ti]
                        dc0 = 304 * gi + 256
                        col0 = GOFF[gi] + 256
                        nc.tensor.matmul(
                            zt[0:108, c0:c0 + ncols],
                            lhsT=wdrxt_sb[:, :, 112 * gi:112 * gi + 108],
                            rhs=xdr[:, :, 0:ncols],
                            start=True, stop=False, perf_mode=DRMODE)
                        nc.tensor.matmul(
                            zt[0:44, c0:c0 + ncols],
                            lhsT=wdrh_sb[0][:, :, dc0:dc0 + 44],
                            rhs=kt_dr[(par, 0)][:, :, 0:ncols],
                            start=False, stop=False, perf_mode=DRMODE)
                        for s in range(2):
                            nc.tensor.matmul(
                                zt[64:108, c0:c0 + ncols],
                                lhsT=wdrh_sb[1][:, s, dc0:dc0 + 44],
                                rhs=kt_dr[(par, 1)][:, s, 0:ncols],
                                start=False, stop=False)
                        nc.tensor.matmul(
                            zt[0:44, c0:c0 + ncols],
                            lhsT=w_slice(0, 2, col0, 44),
                            rhs=kt[(par, 0)][2][:, 0:ncols],
                            start=False, stop=False)
                        nc.tensor.matmul(
                            zt[64:108, c0:c0 + ncols],
                            lhsT=w_slice(1, 2, col0, 44),
                            rhs=kt[(par, 1)][2][:, 0:ncols],
                            start=False, stop=True)
                else:
                    for d in range(2):
                        gate_matmuls(d, zif, zgo, 256, 44, 0 if d == 0 else 64)
                if not do_act:
                    return
                sl = slice(0, 108)
                sif = sifp.tile([128, 1024], mdt, tag="sif")
                nc.scalar.activation(out=sif[sl], in_=zif[sl], func=AF.Sigmoid)
                nc.scalar.activation(out=gc_t[sl, 0:512], in_=zgo[sl, 0:512],
                                     func=AF.Tanh)
                so = sop.tile([128, BS], mdt, tag="so")
                nc.scalar.activation(out=so[sl], in_=zgo[sl, 512:1024],
                                     func=AF.Sigmoid)
                if not do_dve:
                    return
                p1 = p1p.tile([128, 1024], mdt, tag="p1")
                nc.vector.tensor_mul(out=p1[sl], in0=sif[sl], in1=gc_t[sl])
                nc.vector.tensor_add(out=gc_t[sl, 512:1024],
                                     in0=p1[sl, 0:512], in1=p1[sl, 512:1024])
                tcj = tcp.tile([128, BS], mdt, tag="tc")
                nc.scalar.activation(out=tcj[sl], in_=gc_t[sl, 512:1024],
                                     func=AF.Tanh)
                if do_rec:
                    hd0, hd1 = kt[(nxt, 0)][2], kt[(nxt, 1)][2]
                else:
                    hd0 = tcp.tile([128, BS], mdt, tag="hscr")
                    hd1 = hd0
                nc.vector.tensor_mul(out=hd0[0:44], in0=so[0:44],
                                     in1=tcj[0:44])
                # d1: inputs at partitions 64:108, output realigned to 0:44
                # (DVE allows a shifted output when both inputs align).
                nc.vector.tensor_mul(out=hd1[0:44], in0=so[64:108],
                                     in1=tcj[64:108])
                if do_attn:
                    # rows 0:45 include the bias row (=1 in both mix tiles),
                    # so hs[44] = 2 and r[2][44] accumulates 2*sum(e) — the
                    # softmax denominator comes along for free.
                    nc.vector.tensor_add(out=hs[0:45, 2 * BS:3 * BS],
                                         in0=kt[(nxt, 0)][2][0:45],
                                         in1=kt[(nxt, 1)][2][0:45])

            full_group(0, 0)
            if pend_th is not None:
                pend_e16 = attn_score(pend_th)
            full_group(0, 1)
            tail_group()
            full_group(1, 0)
            if pend_th is not None:
                attn_accum(pending_hs, pend_e16)
            full_group(1, 1)
            pending_hs = hs

        if do_attn:
            attn_tail(pending_hs)

        if loop_cm is not None:
            loop_cm.__exit__(None, None, None)

        # ---- tail: hStar = tanh(r / s); logits; softmax ----
        # softmax denominator: r[2][44] = 2 * sum_t e_t (bias-row trick);
        # ACT moves it from partition 44 to partition 0, the *2 is folded
        # into the rs16 copy's scale.
        rs = smp.tile([1, BS], f32, tag="rs")
        if do_attn:
            # partition 44 -> 0 via a one-hot selector matmul (compute
            # engines can't start an access at partition 44)
            srow_ps = atp.tile([1, BS], f32, tag="at")
            nc.tensor.matmul(srow_ps, lhsT=sel44_sb[0:45, 0:1],
                             rhs=r[2][0:45], start=True, stop=True)
            nc.vector.reciprocal(out=rs, in_=srow_ps)
        else:
            srow = smp.tile([1, BS], f32, tag="srow")
            nc.vector.memset(srow, 1.0)   # timing probes: keep 1/s finite
            nc.vector.reciprocal(out=rs, in_=srow)
        rs16 = smp.tile([1, BS], mdt, tag="rs16")
        nc.scalar.activation(out=rs16, in_=rs, func=AF.Copy, scale=2.0)
        rsb = atp.tile([128, BS], f32, tag="at")
        nc.tensor.matmul(rsb, lhsT=ones_sb, rhs=rs16, start=True, stop=True)
        hst = []
        for j in range(3):
            hn = fin.tile([128, BS], f32, tag=f"hn{j}")
            nc.vector.tensor_mul(out=hn, in0=r[j], in1=rsb)
            hj = fin.tile([128, BS], mdt, tag=f"hst{j}")
            nc.scalar.activation(out=hj, in_=hn, func=AF.Tanh)
            hst.append(hj)
        for bt in range(BS // 128):
            fcp = atp.tile([128, NCLS], f32, tag="at")
            for j in range(3):
                nc.tensor.matmul(fcp, lhsT=hst[j][:, bt * 128:(bt + 1) * 128],
                                 rhs=fcw_sb[:, j * NCLS:(j + 1) * NCLS],
                                 start=(j == 0), stop=False)
            nc.tensor.matmul(fcp, lhsT=ones_sb, rhs=fcb_sb, start=False, stop=True)
            mx = fin.tile([128, 1], f32, tag="mx")
            nc.vector.reduce_max(out=mx, in_=fcp, axis=AX.X)
            nmx = fin.tile([128, 1], f32, tag="nmx")
            nc.vector.tensor_scalar_mul(out=nmx, in0=mx, scalar1=-1.0)
            ex = fin.tile([128, NCLS], f32, tag="ex")
            nc.scalar.activation(out=ex, in_=fcp, func=AF.Exp, bias=nmx)
            sm = fin.tile([128, 1], f32, tag="smm")
            nc.vector.reduce_sum(out=sm, in_=ex, axis=AX.X)
            nc.vector.reciprocal(out=sm, in_=sm)
            ot = fin.tile([128, NCLS], f32, tag="ot")
            nc.vector.tensor_scalar_mul(out=ot, in0=ex, scalar1=sm)
            nc.sync.dma_start(out=out_d.ap()[bt * 128:(bt + 1) * 128], in_=ot)

    return nc


def _prep(x, w_ih, w_hh, b_ih, b_hh, conv_w, fc_w, fc_b, np_mdt):
    """Host-side layout prep (shared across cores + per-core x shards).

    Merged contraction rows (640 = 5 k-tiles of 128):
      tile 0: h[0:128]        tile 1: h[128:256]
      tile 2: h[256:300] at parts 0..43, bias (const-1 row) at part 44,
              x[256:300] at parts 64..107, zeros elsewhere
      tile 3: x[0:128]        tile 4: x[128:256]
    """
    bias = (b_ih + b_hh).astype(np.float32)  # [2, 1200]
    wc = np.zeros((2, NK, 128, 1200), np.float32)
    for d in range(2):
        comb = np.zeros((NK * 128, 1200), np.float32)
        comb[0:256] = w_hh[d].T[0:256]
        comb[256:300] = w_hh[d].T[256:300]
        comb[256 + BIASROW] = bias[d]
        comb[256 + XTAIL:256 + XTAIL + 44] = w_ih[d].T[256:300]
        comb[384:512] = w_ih[d].T[0:128]
        comb[512:640] = w_ih[d].T[128:256]
        wc[d] = comb.reshape(NK, 128, 1200)

    def h_pack(vec_or_mat, width):
        """Pack [300(, width)] h-feature data into the 3-tile h k-layout."""
        out = np.zeros((3, 128, width), np.float32)
        v = vec_or_mat.reshape(H, width)
        out[0] = v[0:128]
        out[1] = v[128:256]
        out[2, 0:44] = v[256:300]
        return out

    convp = np.ascontiguousarray(
        h_pack(conv_w, 1).reshape(3, 128).T)          # [128, 3]
    fcw = np.ascontiguousarray(
        h_pack(fc_w.T, NCLS).transpose(1, 0, 2).reshape(128, 3 * NCLS))

    sel44 = np.zeros((128, 1), np.float32)
    sel44[BIASROW, 0] = 1.0
    shared = {
        "wc": wc.astype(np_mdt),
        "convp": convp.astype(np_mdt),
        "fcw": fcw.astype(np_mdt),
        "fcb": fc_b.reshape(1, NCLS).astype(np_mdt),
        "onesrow": np.ones((1, BS), np.float32).astype(np_mdt),
        "sel44": sel44,
    }
    if USE_DR:
        import ml_dtypes
        np_f8 = ml_dtypes.float8_e4m3
        # wdr[d, p, s, 304*g + r] = w_ih[d][300*g + r, 128*s + p]
        wdr = np.zeros((2, 128, 2, WDRW), np.float32)
        wdrh = np.zeros((2, 128, 2, WDRW), np.float32)
        for d in range(2):
            tmp = w_ih[d][:, 0:256].reshape(1200, 2, 128)
            tmph = w_hh[d][:, 0:256].reshape(1200, 2, 128)
            for g in range(4):
                wdr[d, :, :, 304 * g:304 * g + 300] = (
                    tmp[300 * g:300 * g + 300].transpose(2, 1, 0))
                wdrh[d, :, :, 304 * g:304 * g + 300] = (
                    tmph[300 * g:300 * g + 300].transpose(2, 1, 0))
        shared["wdr"] = wdr.astype(np_f8)
        shared["wdrh"] = wdrh.astype(np_f8)
        # combined x gate-tail block: per gate gi (112-col pitch, 16B
        # aligned for DR), cols 0:44 = d0 rows 256:300, 64:108 = d1
        wdrxt = np.zeros((128, 2, 448), np.float32)
        for g in range(4):
            for d in range(2):
                tmp = w_ih[d][:, 0:256].reshape(1200, 2, 128)
                wdrxt[:, :, 112 * g + 64 * d:112 * g + 64 * d + 44] = (
                    tmp[300 * g + 256:300 * g + 300].transpose(2, 1, 0))
        shared["wdrxt"] = wdrxt.astype(np_f8)

    # x: [B, H, T] -> per-core [T, 3, 128, BS]:
    # slot 0 = x[0:128], slot 1 = x[128:256],
    # slot 2 = zeros with x[256:300] at parts 64..107.
    xs = np.ascontiguousarray(np.transpose(x, (2, 1, 0)))  # [T, H, B]
    xp = np.zeros((T, 3, 128, B), np.float32)
    xp[:, 0] = xs[:, 0:128]
    xp[:, 1] = xs[:, 128:256]
    xp[:, 2, XTAIL:XTAIL + 44] = xs[:, 256:300]
    xp = xp.reshape(T, 3, 128, NCORES, BS)
    if USE_DR:
        # xdr[t, p, s, b] = x[t, 128*s + p, b]
        xdr = xs[:, 0:256].reshape(T, 2, 128, NCORES, BS).transpose(0, 2, 1, 3, 4)
    in_maps = []
    for c in range(NCORES):
        m = dict(shared)
        m["xt"] = np.ascontiguousarray(xp[:, :, :, c]).astype(np_mdt)
        if USE_DR:
            import ml_dtypes
            m["xdr"] = np.ascontiguousarray(xdr[:, :, :, c]).astype(
                ml_dtypes.float8_e4m3)
        in_maps.append(m)
    return in_maps


def _np_mdt(mdt_name):
    return np.float16 if mdt_name == "float16" else (
        __import__("ml_dtypes").bfloat16 if mdt_name == "bfloat16" else np.float32)


def _runner(repeat=0, variant="full"):
    key = (MM_DT_NAME, repeat, variant)
    if key not in _CACHE:
        _CACHE[key] = _Runner(_build(MM_DT_NAME, repeat=repeat,
                                     variant=variant), NCORES)
    return _CACHE[key]


def _in_maps(inputs_f32):
    return _prep(*inputs_f32, _np_mdt(MM_DT_NAME))


def _inputs_f32(x, w_ih, w_hh, b_ih, b_hh, conv_w, fc_w, fc_b):
    return [np.asarray(a, np.float32) for a in
            (x, w_ih, w_hh, b_ih, b_hh, conv_w, fc_w, fc_b)]


def kernel(x, w_ih, w_hh, b_ih, b_hh, conv_w, fc_w, fc_b):
    runner = _runner(repeat=0)
    in_maps = _in_maps(_inputs_f32(x, w_ih, w_hh, b_ih, b_hh,
                                   conv_w, fc_w, fc_b))
    results = runner.run(in_maps)
    out = np.concatenate([r["out"] for r in results], axis=0)
    return out.astype(np.float32)


def bench(x, w_ih, w_hh, b_ih, b_hh, conv_w, fc_w, fc_b, iters=5):
    runner = _runner(repeat=0)
    in_maps = _in_maps(_inputs_f32(x, w_ih, w_hh, b_ih, b_hh,
                                   conv_w, fc_w, fc_b))
    return runner.bench(in_maps, iters=iters)


def measure_exec_ns(inputs, r_lo=1, r_hi=301, iters=16):
    """Device execution time of one full forward pass, in ns.

    The axon tunnel adds a fixed ~70-80 ms completion-notification latency
    to every blocking call, independent of what the NEFF does (measured:
    a trivial 4-instruction kernel takes the same wall time as the full
    LSTM).  To measure hardware execution, both builds wrap the whole
    T-step forward in a hardware For_i loop (r_lo vs r_hi iterations,
    identical instruction stream per iteration); the slope
    (min_wall(r_hi) - min_wall(r_lo)) / (r_hi - r_lo) is the steady-state
    on-device time of one forward pass with the constant latency cancelled.
    Samples are interleaved so network drift affects both arms equally.
    """
    import time
    in_maps = _in_maps(_inputs_f32(**inputs) if isinstance(inputs, dict)
                       else _inputs_f32(*inputs))
    runners = {rep: _runner(repeat=rep) for rep in (r_lo, r_hi)}
    dev_in = {rep: runners[rep].put_inputs(in_maps) for rep in (r_lo, r_hi)}
    for rep in (r_lo, r_hi):
        runners[rep].call(dev_in[rep])  # warm
    walls = {r_lo: [], r_hi: []}
    for _ in range(iters):
        for rep in (r_lo, r_hi):
            t0 = time.perf_counter()
            runners[rep].call(dev_in[rep])
            walls[rep].append(time.perf_counter() - t0)
    lo, hi = min(walls[r_lo]), min(walls[r_hi])
    ns = (hi - lo) * 1e9 / (r_hi - r_lo)
    return max(int(ns), 1), walls



# revision 46
# speedup vs baseline: 1.1165x; 1.1165x over previous
"""AttentionLSTM Trainium2 kernel — 8-core data-parallel.

Model (per batch row b): two independent single-direction LSTMs over T=43
steps of x[:, :, t] (H=300 features), hidden states summed, then a
conv-softmax attention over time, tanh, fc(300->80), softmax.

Device mapping per core (512 batch rows).  HW probes (repeat-loop slope
of timing-variant builds, all same-process A/B) put the matmul-only floor
at ~0.67 ms; everything else below is about keeping the other engines and
the recurrence OFF that critical path — the timeline simulator's engine
weights (ACT-bound) did NOT match hardware, which is PE-bound with
dependency stalls:
  - z^T[1200, 512] per (direction, step) via PE matmuls: x[0:256] and
    h[0:256] as fp8 DoubleRow passes, h/x tails + bias via one fp16
    'mix' k-tile pass; M gate-aligned tiles {128,128,44}; fp32 PSUM.
  - h k-tiles are parity double-buffered: step t reads kt[t%2], writes h_t
    into kt[(t+1)%2], so every matmul of a step sees the full h_{t-1}
    (exact LSTM semantics, no Gauss-Seidel staleness).
  - the two directions' 44-row gate tails are PACKED into one PSUM pair
    (d0 at partitions 0:44, d1 at 64:108): ACT/DVE cost scales with
    free-dim columns only, so one elementwise chain covers both tails.
    DR can't write to PSUM col-group 64, so the tail x pass contracts
    BOTH dirs in one DR pass (combined lhsT, out [0:108] at base 0) and
    d1's h contraction runs as two plain fp8 half passes.
  - group order d0j0, d0j1, TAIL, d1j0, d1j1: the tail chain (whose h
    rows feed every mix pass next step) completes mid-step, removing the
    per-step PE stall the tail-at-end ordering caused.
  - PSUM: 3 rotating [128,1024] i|f / g|o pairs + 2 attention banks.
    One fused sigmoid over the i|f pair (ACT instruction count in the
    gate->h chain is expensive on HW), tanh(g)/sigmoid(o) on the g|o
    pair; gate/cell state fp16 (2x/4x DVE modes); c stays in SBUF.
  - per-step x DMAs (xdr + mix x-tails) are prefetched one step ahead.
  - GPSIMD is avoided entirely: Q7 software ops measured ~3x their
    cost-model estimate; fp8 h copies and r += run on DVE instead.
  - attention: hsum tanh emitted in 512-col segments BETWEEN gate groups
    (one big tanh at the step head delays every gate act in the ACT FIFO
    and through them the recurrence); score matmul computes -a via
    negated conv weights so e = exp(a) = 1/sigmoid(-a) - 1 (reciprocal +
    scalar-add on DVE, no ACT copy, no extra multiply); e broadcast by a
    rank-1 PE matmul into PSUM, staged once to SBUF fp16 so the three
    r-product muls hit the fast DVE modes; ALL attention accumulation DVE
    work emitted at the step end, behind the recurrence chain in the DVE
    FIFO; the softmax denominator rides for free in r[2][44] via the mix
    tiles' bias rows (hs[44] = 2 every step), read out post-loop with a
    one-hot selector matmul (engines can't address partition 44 directly).
  - tail: hStar = tanh(r/s), logits = fc(hStar) via PE (batch on PSUM
    partitions), softmax over the 80-class free dim.
"""

import os
import sys

sys.path.insert(0, "/opt/trn_rl_repo")

from contextlib import ExitStack

import numpy as np

import concourse.bass as bass
import concourse.tile as tile
from concourse import mybir
from concourse.bass_utils import run_bass_kernel_spmd  # noqa: F401  (spmd path kept available)

f32 = mybir.dt.float32
AF = mybir.ActivationFunctionType
AX = mybir.AxisListType

_BIRFIX_DONE = False


def _split_multiwaits(bir_json):
    """This walrus build allows one sync-wait per engine instruction; Tile
    attaches one per producer proc. Hoist extras onto standalone
    EventSemaphore instructions inserted just before, same engine queue."""
    import json
    j = json.loads(bir_json.decode() if isinstance(bir_json, bytes) else bir_json)
    for fn in j.get("functions", []):
        for blk in fn.get("blocks", []):
            out = []
            for ins in blk.get("instructions", []):
                si = ins.get("sync_info")
                ow = si.get("on_wait") if si else None
                if ow and len(ow) > 1:
                    for i, w in enumerate(ow[:-1]):
                        out.append({
                            "debug": ins.get("debug", 0),
                            "engine": ins["engine"],
                            "ins": [], "outs": [],
                            "name": f"{ins['name']}_xw{i}",
                            "opcode": "EventSemaphore",
                            "sync_info": {"on_update": [], "on_wait": [w]},
                        })
                    si["on_wait"] = [ow[-1]]
                out.append(ins)
            blk["instructions"] = out
    return json.dumps(j).encode()


def _install_birfix():
    global _BIRFIX_DONE
    if _BIRFIX_DONE:
        return
    from concourse import bass2jax
    orig = bass2jax.compile_bir_kernel

    def patched(bir_json, tmpdir, neff_name="file.neff"):
        return orig(_split_multiwaits(bir_json), tmpdir, neff_name)

    bass2jax.compile_bir_kernel = patched
    _BIRFIX_DONE = True


class _Runner:
    """Compile once; keep the sharded jitted executable + device inputs."""

    def __init__(self, nc, n_cores):
        import jax
        from jax.sharding import Mesh, PartitionSpec
        from jax.experimental.shard_map import shard_map
        from concourse import bass2jax as b2j

        b2j.install_neuronx_cc_hook()
        _install_birfix()
        self.jax = jax
        self.nc = nc
        self.n_cores = n_cores
        part_name = nc.partition_id_tensor.name if nc.partition_id_tensor else None
        in_names, out_names, out_avals, zero_outs = [], [], [], []
        for alloc in nc.m.functions[0].allocations:
            if not isinstance(alloc, mybir.MemoryLocationSet):
                continue
            name = alloc.memorylocations[0].name
            if alloc.kind == "ExternalInput":
                if name != part_name:
                    in_names.append(name)
            elif alloc.kind == "ExternalOutput":
                out_names.append(name)
                shape = tuple(alloc.tensor_shape)
                dtype = mybir.dt.np(alloc.dtype)
                out_avals.append(jax.core.ShapedArray(shape, dtype))
                zero_outs.append(np.zeros(shape, dtype))
        self.in_names = list(in_names)
        self.out_names = out_names
        self.out_avals = out_avals
        self.zero_outs = zero_outs
        n_params = len(in_names)
        n_outs = len(out_avals)
        all_names = in_names + out_names
        if part_name is not None:
            all_names = all_names + [part_name]
        donate = tuple(range(n_params, n_params + n_outs))

        def _body(*args):
            operands = list(args)
            if part_name is not None:
                operands.append(b2j.partition_id_tensor())
            outs = b2j._bass_exec_p.bind(
                *operands,
                out_avals=tuple(out_avals),
                in_names=tuple(all_names),
                out_names=tuple(out_names),
                lowering_input_output_aliases=(),
                sim_require_finite=True,
                sim_require_nnan=True,
                nc=nc,
            )
            return tuple(outs)

        devices = jax.devices()[:n_cores]
        self.mesh = Mesh(np.asarray(devices), ("core",))
        in_specs = (PartitionSpec("core"),) * (n_params + n_outs)
        out_specs = (PartitionSpec("core"),) * n_outs
        self.sharded = jax.jit(
            shard_map(_body, mesh=self.mesh, in_specs=in_specs,
                      out_specs=out_specs, check_rep=False),
            donate_argnums=donate, keep_unused=True)
        self.sharding = jax.sharding.NamedSharding(
            self.mesh, PartitionSpec("core"))

    def put_inputs(self, in_maps):
        jax = self.jax
        concat = [np.concatenate([np.asarray(m[n]) for m in in_maps], axis=0)
                  for n in self.in_names]
        return [jax.device_put(a, self.sharding) for a in concat]

    def call(self, dev_in):
        zeros = [np.zeros((self.n_cores * z.shape[0], *z.shape[1:]), z.dtype)
                 for z in self.zero_outs]
        outs = self.sharded(*dev_in, *zeros)
        self.jax.block_until_ready(outs)
        return outs

    def run(self, in_maps):
        dev_in = self.put_inputs(in_maps)
        outs = self.call(dev_in)
        n = self.n_cores
        return [
            {name: np.asarray(outs[i]).reshape(n, *self.out_avals[i].shape)[c]
             for i, name in enumerate(self.out_names)}
            for c in range(n)
        ]

    def bench(self, in_maps, iters=5):
        import time
        dev_in = self.put_inputs(in_maps)
        self.call(dev_in)  # warm
        times = []
        for _ in range(iters):
            t0 = time.perf_counter()
            self.call(dev_in)
            times.append(time.perf_counter() - t0)
        return times


B, H, T, NCLS = 4096, 300, 43, 80
NCORES = 8
BS = B // NCORES          # 512 batch rows per core
NK = 5                    # k-tiles: [h0:128 | h128:256 | h256:300+bias+xtail | x0:128 | x128:256]
BIASROW = 44              # partition of the bias (constant-1) row in k-tile 2
XTAIL = 64                # x rows 256..300 live at parts 64..108 of k-tile 2
MT = [(0, 128), (128, 128), (256, 44)]    # (moff, msz) per gate, output base partition 0
GOFF = [0, 300, 600, 900]                 # torch gate order i,f,g,o
NPAR = 2                                  # h k-tile phases (3-phase measured no better)
WDRW = 4 * 304                            # fp8 DR weights: 16B-aligned gate pitch

MM_DT_NAME = os.environ.get("LSTM_MM_DT", "float16")
# fp8e4m3 DoubleRow for the x[0:256] contraction: one 256-row matmul pass
# replaces two fp16 128-row passes (20% fewer gate matmul passes; emulated
# end-to-end rel err 8.2e-3 vs the 2e-2 gate).
USE_DR = os.environ.get("LSTM_X_FP8_DR", "1") == "1"
# engine for the per-step fp16->fp8 h copies: GPSIMD frees DVE but Q7
# software ops measured slower on HW than the cost model claims
F8COPY_GPSIMD = os.environ.get("LSTM_F8COPY_GPSIMD", "0") == "1"
# engine for the attention r += accumulation (HW A/B: DVE wins by ~65us —
# Q7 software ops cost ~3x the cost-model estimate on real silicon)
RADD_GPSIMD = os.environ.get("LSTM_RADD_GPSIMD", "0") == "1"
# write h for j<2 directly as fp8 into the DR rhs (skips the fp16 copy;
# attention hsum then reads fp8-quantized h)
H_FP8_DIRECT = os.environ.get("LSTM_H_FP8_DIRECT", "0") == "1"
# emit the attention score right after the tail group instead of after
# the 4th gate group
SCORE_EARLY = os.environ.get("LSTM_SCORE_EARLY", "0") == "1"
# stagger the attention tanh in 512-col segments between gate groups
# (HW A/B: one 1536-col tanh at the step head delays every gate act in
# the ACT FIFO and through them the recurrence; split form is ~60us faster)
TH_SPLIT = os.environ.get("LSTM_TH_SPLIT", "1") == "1"
ZBUFS = int(os.environ.get("LSTM_ZBUFS", "7"))
ZLAYOUT = os.environ.get("LSTM_ZLAYOUT", "pairs")  # banks | pairs | pairs4

_CACHE = {}


def _build(mdt_name, repeat=0, variant="full"):
    # variant: "full" | "no_attn" (skip attention accumulation) |
    # "no_dve" (also skip the c/h elementwise chain) | "mm_only"
    # (matmuls + DMAs only) | "mm_nodma" (matmuls, static rhs) |
    # "mm_n256" (matmuls at N=256).  Non-"full" variants are timing probes.
    do_attn = variant == "full"
    do_dve = variant in ("full", "no_attn", "no_rec")
    do_act = variant not in ("mm_only", "mm_nodma", "mm_n256")
    do_xdma = variant != "mm_nodma"
    do_rec = variant != "no_rec"
    ncols = 256 if variant == "mm_n256" else 512
    mdt = getattr(mybir.dt, mdt_name)
    f8 = mybir.dt.float8e4
    DRMODE = mybir.MatmulPerfMode.DoubleRow
    from concourse.alu_op_type import AluOpType
    nc = bass.Bass(target_bir_lowering=False)

    xt_d = nc.declare_dram_parameter("xt", [T, 3, 128, BS], mdt, isOutput=False)
    if USE_DR:
        xdr_d = nc.declare_dram_parameter("xdr", [T, 128, 2, BS], f8,
                                          isOutput=False)
        wdr_d = nc.declare_dram_parameter("wdr", [2, 128, 2, WDRW], f8,
                                          isOutput=False)
        wdrh_d = nc.declare_dram_parameter("wdrh", [2, 128, 2, WDRW], f8,
                                           isOutput=False)
        # combined x gate-tail weights for the packed tail: per gate,
        # cols 0:44 = d0 tail rows, 64:108 = d1 tail rows (112 pitch)
        wdrxt_d = nc.declare_dram_parameter("wdrxt", [128, 2, 448], f8,
                                            isOutput=False)
    wc_d = nc.declare_dram_parameter("wc", [2, NK, 128, 1200], mdt, isOutput=False)
    conv_d = nc.declare_dram_parameter("convp", [128, 3], mdt, isOutput=False)
    fcw_d = nc.declare_dram_parameter("fcw", [128, 3 * NCLS], mdt, isOutput=False)
    fcb_d = nc.declare_dram_parameter("fcb", [1, NCLS], mdt, isOutput=False)
    ones_d = nc.declare_dram_parameter("onesrow", [1, BS], mdt, isOutput=False)
    sel44_d = nc.declare_dram_parameter("sel44", [128, 1], f32, isOutput=False)
    out_d = nc.declare_dram_parameter("out", [BS, NCLS], f32, isOutput=True)

    with tile.TileContext(nc) as tc, ExitStack() as ctx:
        P = lambda name, bufs, **kw: ctx.enter_context(
            tc.tile_pool(name=name, bufs=bufs, **kw))
        wpool = P("w", 1)
        xpool = P("x", 3)
        # One shared pool for all gate PSUM tiles: 3 x [128,1024] f32 =
        # 6 banks.  With separate zif(bufs=2)/zgo(bufs=1) pools the g|o
        # matmuls of each group waited on the previous group's o-act drain
        # with ~0 margin -> ~1us PE stall per group (~260us/forward).
        # PSUM layout options (HW A/B selects):
        #  banks: ring of 1-bank [128,512] tiles (7 + 1 attention bank)
        #  pairs: ring of 2-bank [128,1024] tiles (3 + 2 attention banks)
        #  pairs4: 4 pair tiles; attention shares the pair ring
        if ZLAYOUT == "banks":
            zp = P("z", ZBUFS if do_attn else 8, space="PSUM")
            atp = P("at", 1, space="PSUM") if do_attn else None
        elif ZLAYOUT == "pairs":
            zp = P("z", 3, space="PSUM")
            atp = P("at", 2, space="PSUM") if do_attn else None
        else:  # pairs4
            zp = P("z", 4, space="PSUM")
            atp = None

        def alloc_z4(pfx):
            """Returns (z4 views, zif_pair_or_None): pairs layouts also
            hand back the [128,1024] i|f tile so sigmoid can fuse over it."""
            if ZLAYOUT == "banks":
                return [zp.tile([128, BS], f32, tag="z", name=f"{pfx}{gi}")
                        for gi in range(4)], None
            zif = zp.tile([128, 1024], f32, tag="z", name=f"{pfx}if")
            zgo = zp.tile([128, 1024], f32, tag="z", name=f"{pfx}go")
            return [zif[:, 0:512], zif[:, 512:1024],
                    zgo[:, 0:512], zgo[:, 512:1024]], zif

        def alloc_att():
            # "banks": ONE tile per step (score uses partition 0, the
            # e-broadcast overwrites it after the sigmoid drains).
            # "pairs": two-buffer ring, fresh tile per use like the old
            # design (attn_score and attn_accum each call alloc_att).
            if atp is not None:
                return atp.tile([128, BS], f32, tag="at", name="at_t")
            return zp.tile([128, 1024], f32, tag="z",
                           name="at_t")[:, 0:512]
        sifp = P("sif", 3)
        sop = P("so", 3)
        gcp = P("gc", 1)
        p1p = P("p1", 3)
        tcp = P("tc", 3)
        hp = P("h", 1)
        hsp = P("hs", 2)
        thp = P("th", 2)
        rp = P("r", 1)
        smp = P("sm", 2)
        tmpp = P("tmp", 2)
        fin = P("fin", 2)

        # ---- weights / constants ----
        wc_sb = {}
        nk_sb = 3 if USE_DR else NK
        for d in range(2):
            for k in range(nk_sb):
                wt = wpool.tile([128, 1200], mdt, tag=f"wc_{d}_{k}")
                nc.sync.dma_start(out=wt, in_=wc_d.ap()[d, k])
                wc_sb[(d, k)] = wt
        wdr_sb = {}
        wdrh_sb = {}
        wdrxt_sb = None
        if USE_DR:
            for d in range(2):
                wt = wpool.tile([128, 2, WDRW], f8, tag=f"wdr_{d}")
                nc.sync.dma_start(out=wt, in_=wdr_d.ap()[d])
                wdr_sb[d] = wt
                wth = wpool.tile([128, 2, WDRW], f8, tag=f"wdrh_{d}")
                nc.sync.dma_start(out=wth, in_=wdrh_d.ap()[d])
                wdrh_sb[d] = wth
            wdrxt_sb = wpool.tile([128, 2, 448], f8, tag="wdrxt")
            nc.sync.dma_start(out=wdrxt_sb, in_=wdrxt_d.ap())
        conv_sb = wpool.tile([128, 3], mdt, tag="conv")
        nc.sync.dma_start(out=conv_sb, in_=conv_d.ap())
        fcw_sb = wpool.tile([128, 3 * NCLS], mdt, tag="fcw")
        nc.sync.dma_start(out=fcw_sb, in_=fcw_d.ap())
        fcb_sb = wpool.tile([1, NCLS], mdt, tag="fcb")
        nc.sync.dma_start(out=fcb_sb, in_=fcb_d.ap())
        ones_sb = wpool.tile([1, 128], mdt, tag="ones")
        nc.vector.memset(ones_sb, 1.0)
        sel44_sb = wpool.tile([128, 1], f32, tag="sel44")
        nc.sync.dma_start(out=sel44_sb, in_=sel44_d.ap())

        # ---- persistent state ----
        # h k-tiles, parity double-buffered: step t reads kt[t%2][d],
        # writes h_t into kt[(t+1)%2][d].
        kt = {}
        kt_dr = {}
        for par in range(NPAR):
            for d in range(2):
                kt[(par, d)] = []
                for j in range(3):
                    t_ = hp.tile([128, BS], mdt, tag=f"kt_{par}_{d}_{j}")
                    nc.vector.memset(t_, 0.0)
                    kt[(par, d)].append(t_)
                nc.sync.dma_start(out=kt[(par, d)][2][BIASROW:BIASROW + 1],
                                  in_=ones_d.ap())
                if USE_DR:
                    td = hp.tile([128, 2, BS], f8, tag=f"ktdr_{par}_{d}")
                    nc.vector.memset(td, 0.0)
                    kt_dr[(par, d)] = td
        # gate/cell state [tanh_g | c]: per (d, j) for the full 128-row
        # groups; ONE shared tile for the packed tails (d0 at partitions
        # 0:44, d1 at 64:108 — matmul col-group alignment needs base 64).
        gc = {}
        for d in range(2):
            for j in range(2):
                g = gcp.tile([128, 1024], mdt, tag=f"gc_{d}_{j}")
                nc.vector.memset(g, 0.0)
                gc[(d, j)] = g
        gc_t = gcp.tile([128, 1024], mdt, tag="gc_t")
        nc.vector.memset(gc_t, 0.0)
        r = []
        for j in range(3):
            rt = rp.tile([128, BS], f32, tag=f"r_{j}")
            nc.vector.memset(rt, 0.0)
            r.append(rt)
        # ssum is accumulated FOR FREE in r[2][44]: the mix k-tiles' bias
        # rows make hs[44] = 2 every step, so r[2][44] = 2 * sum_t e_t.
        # Zero the junk partitions of the two rotating hs buffers once so
        # the packed 1536-col tanh never sees NaN bit patterns.
        # hs rows 45:128 of the j2 segment are junk fed through the packed
        # tanh; nothing downstream reads those partitions of th (score and
        # accum slice [0:45]), so no zeroing is needed.

        def w_slice(d, k, col0, msz):
            return wc_sb[(d, k)][:, col0:col0 + msz]

        def attn_tanh(hs):
            # hs: [128, 1536] packed hsum (j0|j1|j2-tail) from the PREVIOUS
            # step.  One activation covers all three segments.
            th = thp.tile([128, 3 * BS], mdt, tag="th")
            nc.scalar.activation(out=th, in_=hs, func=AF.Tanh)
            return th

        def attn_tanh_seg(hs, th, k):
            # split form: one 512-col segment, interleavable between groups
            # so the big tanh never delays gate activations in the ACT FIFO
            if th is None:
                th = thp.tile([128, 3 * BS], mdt, tag="th")
            nc.scalar.activation(out=th[:, k * BS:k * BS + BS],
                                 in_=hs[:, k * BS:k * BS + BS], func=AF.Tanh)
            return th

        def attn_score(th, at_t):
            # conv weights are negated host-side: the matmul computes -a,
            # so e = exp(a) = (1 - sigmoid(-a)) / sigmoid(-a)
            #              = 1/sigmoid(-a) - 1  — two DVE ops, no multiply.
            a_ps = (at_t if ZLAYOUT == "banks" else alloc_att())[0:1]
            for k in range(3):
                pmax = 45 if k == 2 else 128
                nc.tensor.matmul(a_ps, lhsT=conv_sb[0:pmax, k:k + 1],
                                 rhs=th[0:pmax, k * BS:k * BS + BS],
                                 start=(k == 0), stop=(k == 2))
            sg = smp.tile([1, BS], f32, tag="sg")
            nc.scalar.activation(out=sg, in_=a_ps, func=AF.Sigmoid)
            om = smp.tile([1, BS], f32, tag="om")
            nc.vector.reciprocal(out=om, in_=sg)
            e16 = smp.tile([1, BS], mdt, tag="e16")
            nc.vector.tensor_scalar_add(out=e16, in0=om, scalar1=-1.0)
            return e16

        def attn_accum(hs, e16, at_t):
            eb_ps = at_t if ZLAYOUT == "banks" else alloc_att()
            nc.tensor.matmul(eb_ps, lhsT=ones_sb, rhs=e16, start=True, stop=True)
            # one 1x PSUM read to stage eb in SBUF fp16; the three products
            # then run all-SBUF all-fp16 (4x DVE mode) instead of 1x
            eb = tmpp.tile([128, BS], mdt, tag="eb")
            nc.vector.tensor_copy(out=eb, in_=eb_ps)
            for j in range(3):
                pmax = 45 if j == 2 else 128
                tmp = tmpp.tile([128, BS], mdt, tag=f"tmp{j}")
                nc.vector.tensor_mul(out=tmp[0:pmax],
                                     in0=hs[0:pmax, j * BS:j * BS + BS],
                                     in1=eb[0:pmax])
                radd = nc.gpsimd if RADD_GPSIMD else nc.vector
                radd.tensor_add(out=r[j][0:pmax], in0=r[j][0:pmax],
                                in1=tmp[0:pmax])

        def attn_tail(hs):
            at_t = alloc_att()
            attn_accum(hs, attn_score(attn_tanh(hs), at_t), at_t)

        loop_cm = tc.For_i(0, repeat, 1) if repeat else None
        if loop_cm is not None:
            loop_cm.__enter__()

        pending_hs = None
        if not do_xdma:
            if USE_DR:
                xdr0 = xpool.tile([128, 2, BS], f8, tag="xdr")
                nc.vector.memset(xdr0, 0.0)
            else:
                xa0 = xpool.tile([128, BS], mdt, tag="xa")
                nc.vector.memset(xa0, 0.0)
                xb0 = xpool.tile([128, BS], mdt, tag="xb")
                nc.vector.memset(xb0, 0.0)

        # ---- time loop ----
        # Per-step group order: d0j0, d0j1, TAIL(packed, both dirs), d1j0,
        # d1j1.  The packed tail runs mid-step so its ACT/DVE chain (which
        # produces the h-tail rows every full group's mix pass needs next
        # step) completes ~2 groups before the step ends — the baseline's
        # tail-at-end ordering stalled PE ~1.4us at every step boundary.
        def issue_xdma(t):
            """DMA step t's x into tiles; mix x-tails go into the parity
            tile that step t will read."""
            tiles = {}
            if USE_DR:
                xdr = xpool.tile([128, 2, BS], f8, tag="xdr", name="xdr")
                nc.sync.dma_start(out=xdr, in_=xdr_d.ap()[t])
                tiles["xdr"] = xdr
            else:
                xa = xpool.tile([128, BS], mdt, tag="xa", name="xa")
                nc.sync.dma_start(out=xa, in_=xt_d.ap()[t, 0])
                xb = xpool.tile([128, BS], mdt, tag="xb", name="xb")
                nc.sync.dma_start(out=xb, in_=xt_d.ap()[t, 1])
                tiles["xa"], tiles["xb"] = xa, xb
            for d in range(2):
                nc.sync.dma_start(out=kt[(t % NPAR, d)][2][XTAIL:XTAIL + 44],
                                  in_=xt_d.ap()[t, 2][XTAIL:XTAIL + 44])
            return tiles

        # x for step 0 issued ahead of the loop; inside the loop each step
        # prefetches step t+1 so no PE pass ever waits on DGE latency
        pend_x = issue_xdma(0) if do_xdma else None

        for t in range(T):
            par, nxt = t % NPAR, (t + 1) % NPAR
            xa = xb = xdr = None
            if do_xdma:
                cur_x = pend_x
                if USE_DR:
                    xdr = cur_x["xdr"]
                else:
                    xa, xb = cur_x["xa"], cur_x["xb"]
                pend_x = issue_xdma(t + 1) if t + 1 < T else None
            elif USE_DR:
                xdr = xdr0
            else:
                xa, xb = xa0, xb0
            # previous step's attention tanh: emitted first — its input has
            # been ready since last step, so ACT starts immediately while
            # PE fills the first gate group.  TH_SPLIT staggers it across
            # the step instead (segment after each of the first 3 groups).
            pend = do_attn and pending_hs is not None
            pend_th = (attn_tanh(pending_hs)
                       if (pend and not TH_SPLIT) else None)
            pend_e16 = None

            hs = (hsp.tile([128, 3 * BS], mdt, tag="hs", name="hs")
                  if do_attn else None)

            def gate_matmuls(d, z4, moff, msz, plo):
                """Passes accumulating z for (direction d, row-tile at
                moff..moff+msz), output partitions plo..plo+msz."""
                sl = slice(plo, plo + msz)
                for gi in range(4):
                    zdst = z4[gi][sl, 0:ncols]
                    col0 = GOFF[gi] + moff
                    if USE_DR and plo == 0:
                        dc0 = 304 * gi + moff   # 16B-aligned gate starts
                        nc.tensor.matmul(
                            zdst, lhsT=wdr_sb[d][:, :, dc0:dc0 + msz],
                            rhs=xdr[:, :, 0:ncols],
                            start=True, stop=False, perf_mode=DRMODE)
                        nc.tensor.matmul(
                            zdst, lhsT=wdrh_sb[d][:, :, dc0:dc0 + msz],
                            rhs=kt_dr[(par, d)][:, :, 0:ncols],
                            start=False, stop=False, perf_mode=DRMODE)
                        nc.tensor.matmul(
                            zdst, lhsT=w_slice(d, 2, col0, msz),
                            rhs=kt[(par, d)][2][:, 0:ncols],
                            start=False, stop=True)
                    elif USE_DR:
                        # DoubleRow can't target a nonzero PSUM col-group:
                        # contract the same fp8 tiles half-by-half with
                        # plain passes (fp8 streams at bf16 rate)
                        dc0 = 304 * gi + moff
                        for s in range(2):
                            nc.tensor.matmul(
                                zdst, lhsT=wdr_sb[d][:, s, dc0:dc0 + msz],
                                rhs=xdr[:, s, 0:ncols],
                                start=(s == 0), stop=False)
                        for s in range(2):
                            nc.tensor.matmul(
                                zdst, lhsT=wdrh_sb[d][:, s, dc0:dc0 + msz],
                                rhs=kt_dr[(par, d)][:, s, 0:ncols],
                                start=False, stop=False)
                        nc.tensor.matmul(
                            zdst, lhsT=w_slice(d, 2, col0, msz),
                            rhs=kt[(par, d)][2][:, 0:ncols],
                            start=False, stop=True)
                    else:
                        rhsk = [(0, kt[(par, d)][0]), (1, kt[(par, d)][1]),
                                (2, kt[(par, d)][2]), (3, xa), (4, xb)]
                        for ki, (wk, rtile) in enumerate(rhsk):
                            nc.tensor.matmul(
                                zdst, lhsT=w_slice(d, wk, col0, msz),
                                rhs=rtile[:, 0:ncols],
                                start=(ki == 0), stop=(ki == len(rhsk) - 1))

            def full_group(d, j):
                moff, msz = MT[j]
                sl = slice(0, msz)
                z4, zifp_ = alloc_z4("z")
                gate_matmuls(d, z4, moff, msz, 0)
                if not do_act:
                    return
                sif = sifp.tile([128, 1024], mdt, tag="sif")
                if zifp_ is not None:
                    nc.scalar.activation(out=sif[sl], in_=zifp_[sl],
                                         func=AF.Sigmoid)
                else:
                    nc.scalar.activation(out=sif[sl, 0:512], in_=z4[0][sl],
                                         func=AF.Sigmoid)
                    nc.scalar.activation(out=sif[sl, 512:1024], in_=z4[1][sl],
                                         func=AF.Sigmoid)
                gcj = gc[(d, j)]
                nc.scalar.activation(out=gcj[sl, 0:512], in_=z4[2][sl],
                                     func=AF.Tanh)
                so = sop.tile([128, BS], mdt, tag="so")
                nc.scalar.activation(out=so[sl], in_=z4[3][sl],
                                     func=AF.Sigmoid)
                if not do_dve:
                    return
                # c_new = sig_f * c + sig_i * tanh_g ; h = sig_o * tanh(c)
                p1 = p1p.tile([128, 1024], mdt, tag="p1")
                nc.vector.tensor_mul(out=p1[sl], in0=sif[sl], in1=gcj[sl])
                nc.vector.tensor_add(out=gcj[sl, 512:1024],
                                     in0=p1[sl, 0:512], in1=p1[sl, 512:1024])
                tcj = tcp.tile([128, BS], mdt, tag="tc")
                nc.scalar.activation(out=tcj[sl], in_=gcj[sl, 512:1024],
                                     func=AF.Tanh)
                # h_t lands directly in the next step's rhs k-tile
                if H_FP8_DIRECT and USE_DR:
                    # write h as fp8 straight into the DR rhs: removes the
                    # separate fp8 copy from the recurrence chain; the
                    # attention hsum reads the fp8 planes (small extra
                    # quantization on the attention path only)
                    if do_rec:
                        hdst = kt_dr[(nxt, d)][:, j, :]
                    else:
                        hdst = tcp.tile([128, 2, BS], f8, tag="hscr8")[:, 0, :]
                    nc.vector.tensor_mul(out=hdst[sl], in0=so[sl], in1=tcj[sl])
                    if d == 1 and do_attn:
                        nc.vector.tensor_add(out=hs[:, j * BS:j * BS + BS],
                                             in0=kt_dr[(nxt, 0)][:, j, :],
                                             in1=kt_dr[(nxt, 1)][:, j, :])
                    return
                if do_rec:
                    hdst = kt[(nxt, d)][j]
                else:  # timing probe: same traffic, no recurrence dep
                    hdst = tcp.tile([128, BS], mdt, tag="hscr")
                nc.vector.tensor_mul(out=hdst[sl], in0=so[sl], in1=tcj[sl])
                if USE_DR and do_rec:
                    # fp8 copy feeds next step's DR h matmul; the fp16
                    # original stays for the attention hsum path.
                    eng = nc.gpsimd if F8COPY_GPSIMD else nc.vector
                    eng.tensor_copy(out=kt_dr[(nxt, d)][:, j, :], in_=hdst)
                if d == 1 and do_attn:
                    nc.vector.tensor_add(out=hs[:, j * BS:j * BS + BS],
                                         in0=kt[(nxt, 0)][j],
                                         in1=kt[(nxt, 1)][j])

            def tail_group():
                """Both directions' 44-row gate tails in one PSUM pair:
                d0 at partitions 0:44, d1 at 64:108 (col-group aligned).
                One ACT/DVE chain covers both directions — activation and
                vector cost scale with free-dim columns only, so packing
                partitions halves the tails' elementwise cost."""
                z4, zifp_ = alloc_z4("zt")
                if USE_DR:
                    # 6 passes per gate, one accumulation group per bank:
                    # x for BOTH dirs in one DoubleRow pass (xdr is shared;
                    # the combined lhsT has d0 tail cols at 0:44, d1 at
                    # 64:108 — DR is legal at col-group 0), then h per dir
                    # (d0 as DR at base 0; d1 as two plain fp8 half passes
                    # since DR can't target col-group 64), then the two
                    # fp16 mix passes.
                    for gi in range(4):
                        zt = z4[gi]
                        dc0 = 304 * gi + 256
                        col0 = GOFF[gi] + 256
                        nc.tensor.matmul(
                            zt[0:108, 0:ncols],
                            lhsT=wdrxt_sb[:, :, 112 * gi:112 * gi + 108],
                            rhs=xdr[:, :, 0:ncols],
                            start=True, stop=False, perf_mode=DRMODE)
                        nc.tensor.matmul(
                            zt[0:44, 0:ncols],
                            lhsT=wdrh_sb[0][:, :, dc0:dc0 + 44],
                            rhs=kt_dr[(par, 0)][:, :, 0:ncols],
                            start=False, stop=False, perf_mode=DRMODE)
                        for s in range(2):
                            nc.tensor.matmul(
                                zt[64:108, 0:ncols],
                                lhsT=wdrh_sb[1][:, s, dc0:dc0 + 44],
                                rhs=kt_dr[(par, 1)][:, s, 0:ncols],
                                start=False, stop=False)
                        nc.tensor.matmul(
                            zt[0:44, 0:ncols],
                            lhsT=w_slice(0, 2, col0, 44),
                            rhs=kt[(par, 0)][2][:, 0:ncols],
                            start=False, stop=False)
                        nc.tensor.matmul(
                            zt[64:108, 0:ncols],
                            lhsT=w_slice(1, 2, col0, 44),
                            rhs=kt[(par, 1)][2][:, 0:ncols],
                            start=False, stop=True)
                else:
                    for d in range(2):
                        gate_matmuls(d, z4, 256, 44, 0 if d == 0 else 64)
                if not do_act:
                    return
                sl = slice(0, 108)
                sif = sifp.tile([128, 1024], mdt, tag="sif")
                if zifp_ is not None:
                    nc.scalar.activation(out=sif[sl], in_=zifp_[sl],
                                         func=AF.Sigmoid)
                else:
                    nc.scalar.activation(out=sif[sl, 0:512], in_=z4[0][sl],
                                         func=AF.Sigmoid)
                    nc.scalar.activation(out=sif[sl, 512:1024], in_=z4[1][sl],
                                         func=AF.Sigmoid)
                nc.scalar.activation(out=gc_t[sl, 0:512], in_=z4[2][sl],
                                     func=AF.Tanh)
                so = sop.tile([128, BS], mdt, tag="so")
                nc.scalar.activation(out=so[sl], in_=z4[3][sl],
                                     func=AF.Sigmoid)
                if not do_dve:
                    return
                p1 = p1p.tile([128, 1024], mdt, tag="p1")
                nc.vector.tensor_mul(out=p1[sl], in0=sif[sl], in1=gc_t[sl])
                nc.vector.tensor_add(out=gc_t[sl, 512:1024],
                                     in0=p1[sl, 0:512], in1=p1[sl, 512:1024])
                tcj = tcp.tile([128, BS], mdt, tag="tc")
                nc.scalar.activation(out=tcj[sl], in_=gc_t[sl, 512:1024],
                                     func=AF.Tanh)
                if do_rec:
                    hd0, hd1 = kt[(nxt, 0)][2], kt[(nxt, 1)][2]
                else:
                    hd0 = tcp.tile([128, BS], mdt, tag="hscr")
                    hd1 = hd0
                nc.vector.tensor_mul(out=hd0[0:44], in0=so[0:44],
                                     in1=tcj[0:44])
                # d1: inputs at partitions 64:108, output realigned to 0:44
                # (DVE allows a shifted output when both inputs align).
                nc.vector.tensor_mul(out=hd1[0:44], in0=so[64:108],
                                     in1=tcj[64:108])
                if do_attn:
                    # rows 0:45 include the bias row (=1 in both mix tiles),
                    # so hs[44] = 2 and r[2][44] accumulates 2*sum(e) — the
                    # softmax denominator comes along for free.
                    nc.vector.tensor_add(out=hs[0:45, 2 * BS:3 * BS],
                                         in0=kt[(nxt, 0)][2][0:45],
                                         in1=kt[(nxt, 1)][2][0:45])

            full_group(0, 0)
            if pend and TH_SPLIT:
                pend_th = attn_tanh_seg(pending_hs, None, 0)
            full_group(0, 1)
            if pend and TH_SPLIT:
                attn_tanh_seg(pending_hs, pend_th, 1)
            if pend and TH_SPLIT and SCORE_EARLY:
                attn_tanh_seg(pending_hs, pend_th, 2)
            tail_group()
            if pend and TH_SPLIT and not SCORE_EARLY:
                attn_tanh_seg(pending_hs, pend_th, 2)
            if pend and SCORE_EARLY:
                at_t = alloc_att()
                if TH_SPLIT:
                    pend_e16 = attn_score(pend_th, at_t)
            full_group(1, 0)
            if pend and not SCORE_EARLY:
                at_t = alloc_att()
                if TH_SPLIT:
                    pend_e16 = attn_score(pend_th, at_t)
            full_group(1, 1)
            # attention accumulation LAST: its DVE ops sit behind every
            # h-chain op of this step in the DVE FIFO, so they fill the
            # step boundary instead of delaying the recurrence.
            if pend:
                if not TH_SPLIT:
                    pend_e16 = attn_score(pend_th, at_t)
                attn_accum(pending_hs, pend_e16, at_t)
            pending_hs = hs

        if do_attn:
            attn_tail(pending_hs)

        if loop_cm is not None:
            loop_cm.__exit__(None, None, None)

        # ---- tail: hStar = tanh(r / s); logits; softmax ----
        # softmax denominator: r[2][44] = 2 * sum_t e_t (bias-row trick);
        # ACT moves it from partition 44 to partition 0, the *2 is folded
        # into the rs16 copy's scale.
        rs = smp.tile([1, BS], f32, tag="rs")
        if do_attn:
            # partition 44 -> 0 via a one-hot selector matmul (compute
            # engines can't start an access at partition 44)
            srow_ps = zp.tile([128, BS], f32, tag="z", name="srow_ps")[0:1]
            nc.tensor.matmul(srow_ps, lhsT=sel44_sb[0:45, 0:1],
                             rhs=r[2][0:45], start=True, stop=True)
            nc.vector.reciprocal(out=rs, in_=srow_ps)
        else:
            srow = smp.tile([1, BS], f32, tag="srow")
            nc.vector.memset(srow, 1.0)   # timing probes: keep 1/s finite
            nc.vector.reciprocal(out=rs, in_=srow)
        rs16 = smp.tile([1, BS], mdt, tag="rs16")
        nc.scalar.activation(out=rs16, in_=rs, func=AF.Copy, scale=2.0)
        def _att_ps(shape):
            zt = zp.tile([128, BS], f32, tag="z", name="attps")
            return zt[0:shape[0], 0:shape[1]]
        rsb = _att_ps([128, BS])
        nc.tensor.matmul(rsb, lhsT=ones_sb, rhs=rs16, start=True, stop=True)
        hst = []
        for j in range(3):
            hn = fin.tile([128, BS], f32, tag=f"hn{j}")
            nc.vector.tensor_mul(out=hn, in0=r[j], in1=rsb)
            hj = fin.tile([128, BS], mdt, tag=f"hst{j}")
            nc.scalar.activation(out=hj, in_=hn, func=AF.Tanh)
            hst.append(hj)
        for bt in range(BS // 128):
            fcp = _att_ps([128, NCLS])
            for j in range(3):
                nc.tensor.matmul(fcp, lhsT=hst[j][:, bt * 128:(bt + 1) * 128],
                                 rhs=fcw_sb[:, j * NCLS:(j + 1) * NCLS],
                                 start=(j == 0), stop=False)
            nc.tensor.matmul(fcp, lhsT=ones_sb, rhs=fcb_sb, start=False, stop=True)
            mx = fin.tile([128, 1], f32, tag="mx")
            nc.vector.reduce_max(out=mx, in_=fcp, axis=AX.X)
            nmx = fin.tile([128, 1], f32, tag="nmx")
            nc.vector.tensor_scalar_mul(out=nmx, in0=mx, scalar1=-1.0)
            ex = fin.tile([128, NCLS], f32, tag="ex")
            nc.scalar.activation(out=ex, in_=fcp, func=AF.Exp, bias=nmx)
            sm = fin.tile([128, 1], f32, tag="smm")
            nc.vector.reduce_sum(out=sm, in_=ex, axis=AX.X)
            nc.vector.reciprocal(out=sm, in_=sm)
            ot = fin.tile([128, NCLS], f32, tag="ot")
            nc.vector.tensor_scalar_mul(out=ot, in0=ex, scalar1=sm)
            nc.sync.dma_start(out=out_d.ap()[bt * 128:(bt + 1) * 128], in_=ot)

    return nc


def _prep(x, w_ih, w_hh, b_ih, b_hh, conv_w, fc_w, fc_b, np_mdt):
    """Host-side layout prep (shared across cores + per-core x shards).

    Merged contraction rows (640 = 5 k-tiles of 128):
      tile 0: h[0:128]        tile 1: h[128:256]
      tile 2: h[256:300] at parts 0..43, bias (const-1 row) at part 44,
              x[256:300] at parts 64..107, zeros elsewhere
      tile 3: x[0:128]        tile 4: x[128:256]
    """
    bias = (b_ih + b_hh).astype(np.float32)  # [2, 1200]
    wc = np.zeros((2, NK, 128, 1200), np.float32)
    for d in range(2):
        comb = np.zeros((NK * 128, 1200), np.float32)
        comb[0:256] = w_hh[d].T[0:256]
        comb[256:300] = w_hh[d].T[256:300]
        comb[256 + BIASROW] = bias[d]
        comb[256 + XTAIL:256 + XTAIL + 44] = w_ih[d].T[256:300]
        comb[384:512] = w_ih[d].T[0:128]
        comb[512:640] = w_ih[d].T[128:256]
        wc[d] = comb.reshape(NK, 128, 1200)

    def h_pack(vec_or_mat, width):
        """Pack [300(, width)] h-feature data into the 3-tile h k-layout."""
        out = np.zeros((3, 128, width), np.float32)
        v = vec_or_mat.reshape(H, width)
        out[0] = v[0:128]
        out[1] = v[128:256]
        out[2, 0:44] = v[256:300]
        return out

    # conv NEGATED: the score matmul computes -a so the in-loop exp trick
    # is e = 1/sigmoid(-a) - 1 (two DVE ops)
    convp = np.ascontiguousarray(
        -h_pack(conv_w, 1).reshape(3, 128).T)         # [128, 3]
    fcw = np.ascontiguousarray(
        h_pack(fc_w.T, NCLS).transpose(1, 0, 2).reshape(128, 3 * NCLS))

    sel44 = np.zeros((128, 1), np.float32)
    sel44[BIASROW, 0] = 1.0
    shared = {
        "wc": wc.astype(np_mdt),
        "convp": convp.astype(np_mdt),
        "fcw": fcw.astype(np_mdt),
        "fcb": fc_b.reshape(1, NCLS).astype(np_mdt),
        "onesrow": np.ones((1, BS), np.float32).astype(np_mdt),
        "sel44": sel44,
    }
    if USE_DR:
        import ml_dtypes
        np_f8 = ml_dtypes.float8_e4m3
        # wdr[d, p, s, 304*g + r] = w_ih[d][300*g + r, 128*s + p]
        wdr = np.zeros((2, 128, 2, WDRW), np.float32)
        wdrh = np.zeros((2, 128, 2, WDRW), np.float32)
        for d in range(2):
            tmp = w_ih[d][:, 0:256].reshape(1200, 2, 128)
            tmph = w_hh[d][:, 0:256].reshape(1200, 2, 128)
            for g in range(4):
                wdr[d, :, :, 304 * g:304 * g + 300] = (
                    tmp[300 * g:300 * g + 300].transpose(2, 1, 0))
                wdrh[d, :, :, 304 * g:304 * g + 300] = (
                    tmph[300 * g:300 * g + 300].transpose(2, 1, 0))
        shared["wdr"] = wdr.astype(np_f8)
        shared["wdrh"] = wdrh.astype(np_f8)
        # combined x gate-tail block: per gate gi (112-col pitch, 16B
        # aligned for DR), cols 0:44 = d0 rows 256:300, 64:108 = d1
        wdrxt = np.zeros((128, 2, 448), np.float32)
        for g in range(4):
            for d in range(2):
                tmp = w_ih[d][:, 0:256].reshape(1200, 2, 128)
                wdrxt[:, :, 112 * g + 64 * d:112 * g + 64 * d + 44] = (
                    tmp[300 * g + 256:300 * g + 300].transpose(2, 1, 0))
        shared["wdrxt"] = wdrxt.astype(np_f8)

    # x: [B, H, T] -> per-core [T, 3, 128, BS]:
    # slot 0 = x[0:128], slot 1 = x[128:256],
    # slot 2 = zeros with x[256:300] at parts 64..107.
    xs = np.ascontiguousarray(np.transpose(x, (2, 1, 0)))  # [T, H, B]
    xp = np.zeros((T, 3, 128, B), np.float32)
    xp[:, 0] = xs[:, 0:128]
    xp[:, 1] = xs[:, 128:256]
    xp[:, 2, XTAIL:XTAIL + 44] = xs[:, 256:300]
    xp = xp.reshape(T, 3, 128, NCORES, BS)
    if USE_DR:
        # xdr[t, p, s, b] = x[t, 128*s + p, b]
        xdr = xs[:, 0:256].reshape(T, 2, 128, NCORES, BS).transpose(0, 2, 1, 3, 4)
    in_maps = []
    for c in range(NCORES):
        m = dict(shared)
        m["xt"] = np.ascontiguousarray(xp[:, :, :, c]).astype(np_mdt)
        if USE_DR:
            import ml_dtypes
            m["xdr"] = np.ascontiguousarray(xdr[:, :, :, c]).astype(
                ml_dtypes.float8_e4m3)
        in_maps.append(m)
    return in_maps


def _np_mdt(mdt_name):
    return np.float16 if mdt_name == "float16" else (
        __import__("ml_dtypes").bfloat16 if mdt_name == "bfloat16" else np.float32)


def _runner(repeat=0, variant="full"):
    key = (MM_DT_NAME, repeat, variant)
    if key not in _CACHE:
        _CACHE[key] = _Runner(_build(MM_DT_NAME, repeat=repeat,
                                     variant=variant), NCORES)
    return _CACHE[key]


def _in_maps(inputs_f32):
    return _prep(*inputs_f32, _np_mdt(MM_DT_NAME))


def _inputs_f32(x, w_ih, w_hh, b_ih, b_hh, conv_w, fc_w, fc_b):
    return [np.asarray(a, np.float32) for a in
            (x, w_ih, w_hh, b_ih, b_hh, conv_w, fc_w, fc_b)]


def kernel(x, w_ih, w_hh, b_ih, b_hh, conv_w, fc_w, fc_b):
    runner = _runner(repeat=0)
    in_maps = _in_maps(_inputs_f32(x, w_ih, w_hh, b_ih, b_hh,
                                   conv_w, fc_w, fc_b))
    results = runner.run(in_maps)
    out = np.concatenate([r["out"] for r in results], axis=0)
    return out.astype(np.float32)


def bench(x, w_ih, w_hh, b_ih, b_hh, conv_w, fc_w, fc_b, iters=5):
    runner = _runner(repeat=0)
    in_maps = _in_maps(_inputs_f32(x, w_ih, w_hh, b_ih, b_hh,
                                   conv_w, fc_w, fc_b))
    return runner.bench(in_maps, iters=iters)


def measure_exec_ns(inputs, r_lo=1, r_hi=301, iters=16):
    """Device execution time of one full forward pass, in ns.

    The axon tunnel adds a fixed ~70-80 ms completion-notification latency
    to every blocking call, independent of what the NEFF does (measured:
    a trivial 4-instruction kernel takes the same wall time as the full
    LSTM).  To measure hardware execution, both builds wrap the whole
    T-step forward in a hardware For_i loop (r_lo vs r_hi iterations,
    identical instruction stream per iteration); the slope
    (median_wall(r_hi) - median_wall(r_lo)) / (r_hi - r_lo) is the
    steady-state on-device time of one forward pass with the constant
    latency cancelled.  Samples are interleaved so network drift affects
    both arms equally; the first two samples per arm are discarded (the
    call right after the warm call consistently measures ~25 ms fast —
    a tunnel batching artifact that corrupted the old min() estimator
    asymmetrically) and the median kills remaining outliers.
    """
    import statistics
    import time
    in_maps = _in_maps(_inputs_f32(**inputs) if isinstance(inputs, dict)
                       else _inputs_f32(*inputs))
    runners = {rep: _runner(repeat=rep) for rep in (r_lo, r_hi)}
    dev_in = {rep: runners[rep].put_inputs(in_maps) for rep in (r_lo, r_hi)}
    for rep in (r_lo, r_hi):
        runners[rep].call(dev_in[rep])  # warm
    walls = {r_lo: [], r_hi: []}
    for i in range(iters):
        for rep in (r_lo, r_hi):
            t0 = time.perf_counter()
            runners[rep].call(dev_in[rep])
            walls[rep].append(time.perf_counter() - t0)
        if i == 1:
            for rep in (r_lo, r_hi):
                walls[rep].clear()
    lo = statistics.median(walls[r_lo])
    hi = statistics.median(walls[r_hi])
    ns = (hi - lo) * 1e9 / (r_hi - r_lo)
    return max(int(ns), 1), walls



# revision 52
# speedup vs baseline: 1.2336x; 1.1049x over previous
"""AttentionLSTM Trainium2 kernel — 8-core data-parallel.

Model (per batch row b): two independent single-direction LSTMs over T=43
steps of x[:, :, t] (H=300 features), hidden states summed, then a
conv-softmax attention over time, tanh, fc(300->80), softmax.

Device mapping per core (512 batch rows).  HW probes (repeat-loop slope
of timing-variant builds, all same-process A/B) put the matmul-only floor
at ~0.67 ms; everything else below is about keeping the other engines and
the recurrence OFF that critical path — the timeline simulator's engine
weights (ACT-bound) did NOT match hardware, which is PE-bound with
dependency stalls:
  - z^T[1200, 512] per (direction, step) via PE matmuls: x[0:256] and
    h[0:256] as fp8 DoubleRow passes, h/x tails + bias via one fp16
    'mix' k-tile pass; M gate-aligned tiles {128,128,44}; fp32 PSUM.
  - h k-tiles are parity double-buffered: step t reads kt[t%2], writes h_t
    into kt[(t+1)%2], so every matmul of a step sees the full h_{t-1}
    (exact LSTM semantics, no Gauss-Seidel staleness).
  - the two directions' 44-row gate tails are PACKED into one PSUM pair
    (d0 at partitions 0:44, d1 at 64:108): ACT/DVE cost scales with
    free-dim columns only, so one elementwise chain covers both tails.
    DR can't write to PSUM col-group 64, so the tail x pass contracts
    BOTH dirs in one DR pass (combined lhsT, out [0:108] at base 0) and
    d1's h contraction runs as two plain fp8 half passes.
  - group order d0j0, d0j1, TAIL, d1j0, d1j1: the tail chain (whose h
    rows feed every mix pass next step) completes mid-step, removing the
    per-step PE stall the tail-at-end ordering caused.
  - PSUM: 3 rotating [128,1024] i|f / g|o pairs + 2 attention banks.
    One fused sigmoid over the i|f pair (ACT instruction count in the
    gate->h chain is expensive on HW), tanh(g)/sigmoid(o) on the g|o
    pair; gate/cell state fp16 (2x/4x DVE modes); c stays in SBUF.
  - per-step x DMAs (xdr + mix x-tails) are prefetched one step ahead.
  - GPSIMD is avoided entirely: Q7 software ops measured ~3x their
    cost-model estimate; fp8 h copies and r += run on DVE instead.
  - attention: hsum tanh emitted in 512-col segments BETWEEN gate groups
    (one big tanh at the step head delays every gate act in the ACT FIFO
    and through them the recurrence); score matmul computes -a via
    negated conv weights so e = exp(a) = 1/sigmoid(-a) - 1 (reciprocal +
    scalar-add on DVE, no ACT copy, no extra multiply); e broadcast by a
    rank-1 PE matmul into PSUM, staged once to SBUF fp16 so the three
    r-product muls hit the fast DVE modes; ALL attention accumulation DVE
    work emitted at the step end, behind the recurrence chain in the DVE
    FIFO; the softmax denominator rides for free in r[2][44] via the mix
    tiles' bias rows (hs[44] = 2 every step), read out post-loop with a
    one-hot selector matmul (engines can't address partition 44 directly).
  - tail: hStar = tanh(r/s), logits = fc(hStar) via PE (batch on PSUM
    partitions), softmax over the 80-class free dim.
"""

import os
import sys

sys.path.insert(0, "/opt/trn_rl_repo")

from contextlib import ExitStack

import numpy as np

import concourse.bass as bass
import concourse.tile as tile
from concourse import mybir
from concourse.bass_utils import run_bass_kernel_spmd  # noqa: F401  (spmd path kept available)

f32 = mybir.dt.float32
AF = mybir.ActivationFunctionType
AX = mybir.AxisListType

_BIRFIX_DONE = False


def _split_multiwaits(bir_json):
    """This walrus build allows one sync-wait per engine instruction; Tile
    attaches one per producer proc. Hoist extras onto standalone
    EventSemaphore instructions inserted just before, same engine queue."""
    import json
    j = json.loads(bir_json.decode() if isinstance(bir_json, bytes) else bir_json)
    for fn in j.get("functions", []):
        for blk in fn.get("blocks", []):
            out = []
            for ins in blk.get("instructions", []):
                si = ins.get("sync_info")
                ow = si.get("on_wait") if si else None
                if ow and len(ow) > 1:
                    for i, w in enumerate(ow[:-1]):
                        out.append({
                            "debug": ins.get("debug", 0),
                            "engine": ins["engine"],
                            "ins": [], "outs": [],
                            "name": f"{ins['name']}_xw{i}",
                            "opcode": "EventSemaphore",
                            "sync_info": {"on_update": [], "on_wait": [w]},
                        })
                    si["on_wait"] = [ow[-1]]
                out.append(ins)
            blk["instructions"] = out
    return json.dumps(j).encode()


def _install_birfix():
    global _BIRFIX_DONE
    if _BIRFIX_DONE:
        return
    from concourse import bass2jax
    orig = bass2jax.compile_bir_kernel

    def patched(bir_json, tmpdir, neff_name="file.neff"):
        return orig(_split_multiwaits(bir_json), tmpdir, neff_name)

    bass2jax.compile_bir_kernel = patched
    _BIRFIX_DONE = True


class _Runner:
    """Compile once; keep the sharded jitted executable + device inputs."""

    def __init__(self, nc, n_cores):
        import jax
        from jax.sharding import Mesh, PartitionSpec
        from jax.experimental.shard_map import shard_map
        from concourse import bass2jax as b2j

        b2j.install_neuronx_cc_hook()
        _install_birfix()
        self.jax = jax
        self.nc = nc
        self.n_cores = n_cores
        part_name = nc.partition_id_tensor.name if nc.partition_id_tensor else None
        in_names, out_names, out_avals, zero_outs = [], [], [], []
        for alloc in nc.m.functions[0].allocations:
            if not isinstance(alloc, mybir.MemoryLocationSet):
                continue
            name = alloc.memorylocations[0].name
            if alloc.kind == "ExternalInput":
                if name != part_name:
                    in_names.append(name)
            elif alloc.kind == "ExternalOutput":
                out_names.append(name)
                shape = tuple(alloc.tensor_shape)
                dtype = mybir.dt.np(alloc.dtype)
                out_avals.append(jax.core.ShapedArray(shape, dtype))
                zero_outs.append(np.zeros(shape, dtype))
        self.in_names = list(in_names)
        self.out_names = out_names
        self.out_avals = out_avals
        self.zero_outs = zero_outs
        n_params = len(in_names)
        n_outs = len(out_avals)
        all_names = in_names + out_names
        if part_name is not None:
            all_names = all_names + [part_name]
        donate = tuple(range(n_params, n_params + n_outs))

        def _body(*args):
            operands = list(args)
            if part_name is not None:
                operands.append(b2j.partition_id_tensor())
            outs = b2j._bass_exec_p.bind(
                *operands,
                out_avals=tuple(out_avals),
                in_names=tuple(all_names),
                out_names=tuple(out_names),
                lowering_input_output_aliases=(),
                sim_require_finite=True,
                sim_require_nnan=True,
                nc=nc,
            )
            return tuple(outs)

        devices = jax.devices()[:n_cores]
        self.mesh = Mesh(np.asarray(devices), ("core",))
        in_specs = (PartitionSpec("core"),) * (n_params + n_outs)
        out_specs = (PartitionSpec("core"),) * n_outs
        self.sharded = jax.jit(
            shard_map(_body, mesh=self.mesh, in_specs=in_specs,
                      out_specs=out_specs, check_rep=False),
            donate_argnums=donate, keep_unused=True)
        self.sharding = jax.sharding.NamedSharding(
            self.mesh, PartitionSpec("core"))

    def put_inputs(self, in_maps):
        jax = self.jax
        concat = [np.concatenate([np.asarray(m[n]) for m in in_maps], axis=0)
                  for n in self.in_names]
        return [jax.device_put(a, self.sharding) for a in concat]

    def call(self, dev_in):
        zeros = [np.zeros((self.n_cores * z.shape[0], *z.shape[1:]), z.dtype)
                 for z in self.zero_outs]
        outs = self.sharded(*dev_in, *zeros)
        self.jax.block_until_ready(outs)
        return outs

    def run(self, in_maps):
        dev_in = self.put_inputs(in_maps)
        outs = self.call(dev_in)
        n = self.n_cores
        return [
            {name: np.asarray(outs[i]).reshape(n, *self.out_avals[i].shape)[c]
             for i, name in enumerate(self.out_names)}
            for c in range(n)
        ]

    def bench(self, in_maps, iters=5):
        import time
        dev_in = self.put_inputs(in_maps)
        self.call(dev_in)  # warm
        times = []
        for _ in range(iters):
            t0 = time.perf_counter()
            self.call(dev_in)
            times.append(time.perf_counter() - t0)
        return times


B, H, T, NCLS = 4096, 300, 43, 80
NCORES = 8
BS = B // NCORES          # 512 batch rows per core
NK = 5                    # k-tiles: [h0:128 | h128:256 | h256:300+bias+xtail | x0:128 | x128:256]
BIASROW = 44              # partition of the bias (constant-1) row in k-tile 2
XTAIL = 64                # x rows 256..300 live at parts 64..108 of k-tile 2
MT = [(0, 128), (128, 128), (256, 44)]    # (moff, msz) per gate, output base partition 0
GOFF = [0, 300, 600, 900]                 # torch gate order i,f,g,o
NPAR = 2                                  # h k-tile phases (3-phase measured no better)
WDRW = 4 * 304                            # fp8 DR weights: 16B-aligned gate pitch

MM_DT_NAME = os.environ.get("LSTM_MM_DT", "float16")
# fp8e4m3 DoubleRow for the x[0:256] contraction: one 256-row matmul pass
# replaces two fp16 128-row passes (20% fewer gate matmul passes; emulated
# end-to-end rel err 8.2e-3 vs the 2e-2 gate).
USE_DR = os.environ.get("LSTM_X_FP8_DR", "1") == "1"
# engine for the per-step fp16->fp8 h copies: GPSIMD frees DVE but Q7
# software ops measured slower on HW than the cost model claims
F8COPY_GPSIMD = os.environ.get("LSTM_F8COPY_GPSIMD", "0") == "1"
# engine for the attention r += accumulation (HW A/B: DVE wins by ~65us —
# Q7 software ops cost ~3x the cost-model estimate on real silicon)
RADD_GPSIMD = os.environ.get("LSTM_RADD_GPSIMD", "0") == "1"
# write h for j<2 directly as fp8 into the DR rhs (skips the fp16 copy;
# attention hsum then reads fp8-quantized h)
H_FP8_DIRECT = os.environ.get("LSTM_H_FP8_DIRECT", "0") == "1"
# emit the attention score right after the tail group instead of after
# the 4th gate group
SCORE_EARLY = os.environ.get("LSTM_SCORE_EARLY", "0") == "1"
# emit the attention accumulation between the last two gate groups
# instead of after them
ACC_MID = os.environ.get("LSTM_ACC_MID", "0") == "1"
# allocate the LAST group's g|o PSUM tile before its i|f tile: the g|o
# tile drains latest (two acts after the mix pass), and in alloc order it
# was the ring tenant blocking the next step's second group's i|f tile
# (HW A/B: ~1% better)
ZSWAP = os.environ.get("LSTM_ZSWAP", "1") == "1"
# merge the two per-direction fp8 h copies into one 1024-col copy
F8MERGE = os.environ.get("LSTM_F8MERGE", "0") == "1"
# extend the swap to d1j0 as well (next step's FIRST tile then waits on an
# i|f tile, which drains one activation earlier than g|o)
ZSWAP_D1J0 = os.environ.get("LSTM_ZSWAP_D1J0", "0") == "1"
# pipeline the attention front-end to the step HEAD: th segments are
# computed in the PRODUCING step (right after each hsum segment lands),
# score/exp/e-broadcast run at the head of the next step so their PSUM
# tiles drain early — freeing the 2 attention banks for a 4th z pair
ATT_HEAD = os.environ.get("LSTM_ATT_HEAD", "0") == "1"
# stagger the attention tanh in 512-col segments between gate groups
# (HW A/B: one 1536-col tanh at the step head delays every gate act in
# the ACT FIFO and through them the recurrence; split form is ~60us faster)
TH_SPLIT = os.environ.get("LSTM_TH_SPLIT", "1") == "1"
ZBUFS = int(os.environ.get("LSTM_ZBUFS", "7"))
ZLAYOUT = os.environ.get("LSTM_ZLAYOUT", "pairs")  # banks | pairs | pairs4

_CACHE = {}


def _build(mdt_name, repeat=0, variant="full"):
    # variant: "full" | "no_attn" (skip attention accumulation) |
    # "no_dve" (also skip the c/h elementwise chain) | "mm_only"
    # (matmuls + DMAs only) | "mm_nodma" (matmuls, static rhs) |
    # "mm_n256" (matmuls at N=256).  Non-"full" variants are timing probes.
    do_attn = variant == "full"
    do_dve = variant in ("full", "no_attn", "no_rec")
    do_act = variant not in ("mm_only", "mm_nodma", "mm_n256")
    do_xdma = variant != "mm_nodma"
    do_rec = variant != "no_rec"
    ncols = 256 if variant == "mm_n256" else 512
    mdt = getattr(mybir.dt, mdt_name)
    f8 = mybir.dt.float8e4
    DRMODE = mybir.MatmulPerfMode.DoubleRow
    from concourse.alu_op_type import AluOpType
    nc = bass.Bass(target_bir_lowering=False)

    xt_d = nc.declare_dram_parameter("xt", [T, 3, 128, BS], mdt, isOutput=False)
    if USE_DR:
        xdr_d = nc.declare_dram_parameter("xdr", [T, 128, 2, BS], f8,
                                          isOutput=False)
        wdr_d = nc.declare_dram_parameter("wdr", [2, 128, 2, WDRW], f8,
                                          isOutput=False)
        wdrh_d = nc.declare_dram_parameter("wdrh", [2, 128, 2, WDRW], f8,
                                           isOutput=False)
        # combined x gate-tail weights for the packed tail: per gate,
        # cols 0:44 = d0 tail rows, 64:108 = d1 tail rows (112 pitch)
        wdrxt_d = nc.declare_dram_parameter("wdrxt", [128, 2, 448], f8,
                                            isOutput=False)
    wc_d = nc.declare_dram_parameter("wc", [2, NK, 128, 1200], mdt, isOutput=False)
    conv_d = nc.declare_dram_parameter("convp", [128, 3], mdt, isOutput=False)
    fcw_d = nc.declare_dram_parameter("fcw", [128, 3 * NCLS], mdt, isOutput=False)
    fcb_d = nc.declare_dram_parameter("fcb", [1, NCLS], mdt, isOutput=False)
    ones_d = nc.declare_dram_parameter("onesrow", [1, BS], mdt, isOutput=False)
    sel44_d = nc.declare_dram_parameter("sel44", [128, 1], f32, isOutput=False)
    out_d = nc.declare_dram_parameter("out", [BS, NCLS], f32, isOutput=True)

    with tile.TileContext(nc) as tc, ExitStack() as ctx:
        P = lambda name, bufs, **kw: ctx.enter_context(
            tc.tile_pool(name=name, bufs=bufs, **kw))
        wpool = P("w", 1)
        xpool = P("x", 3)
        # One shared pool for all gate PSUM tiles: 3 x [128,1024] f32 =
        # 6 banks.  With separate zif(bufs=2)/zgo(bufs=1) pools the g|o
        # matmuls of each group waited on the previous group's o-act drain
        # with ~0 margin -> ~1us PE stall per group (~260us/forward).
        # PSUM layout options (HW A/B selects):
        #  banks: ring of 1-bank [128,512] tiles (7 + 1 attention bank)
        #  pairs: ring of 2-bank [128,1024] tiles (3 + 2 attention banks)
        #  pairs4: 4 pair tiles; attention shares the pair ring
        if ATT_HEAD and do_attn:
            zp = P("z", 4, space="PSUM")
            atp = None
        elif ZLAYOUT == "banks":
            zp = P("z", ZBUFS if do_attn else 8, space="PSUM")
            atp = P("at", 1, space="PSUM") if do_attn else None
        elif ZLAYOUT == "pairs":
            zp = P("z", 3, space="PSUM")
            atp = P("at", 2, space="PSUM") if do_attn else None
        else:  # pairs4
            zp = P("z", 4, space="PSUM")
            atp = None

        def alloc_z4(pfx):
            """Returns (z4 views, zif_pair_or_None): pairs layouts also
            hand back the [128,1024] i|f tile so sigmoid can fuse over it."""
            if ZLAYOUT == "banks":
                return [zp.tile([128, BS], f32, tag="z", name=f"{pfx}{gi}")
                        for gi in range(4)], None
            if pfx == "zswap":
                zgo = zp.tile([128, 1024], f32, tag="z", name=f"{pfx}go")
                zif = zp.tile([128, 1024], f32, tag="z", name=f"{pfx}if")
            else:
                zif = zp.tile([128, 1024], f32, tag="z", name=f"{pfx}if")
                zgo = zp.tile([128, 1024], f32, tag="z", name=f"{pfx}go")
            return [zif[:, 0:512], zif[:, 512:1024],
                    zgo[:, 0:512], zgo[:, 512:1024]], zif

        def alloc_att():
            # "banks": ONE tile per step (score uses partition 0, the
            # e-broadcast overwrites it after the sigmoid drains).
            # "pairs": two-buffer ring, fresh tile per use like the old
            # design (attn_score and attn_accum each call alloc_att).
            if atp is not None:
                return atp.tile([128, BS], f32, tag="at", name="at_t")
            return zp.tile([128, 1024], f32, tag="z",
                           name="at_t")[:, 0:512]

        # th tiles owned by the ATT_HEAD pipeline: built in the producing
        # step, consumed by attn_front at the head of the next step
        sifp = P("sif", 3)
        sop = P("so", 3)
        gcp = P("gc", 1)
        p1p = P("p1", 3)
        tcp = P("tc", 3)
        hp = P("h", 1)
        hsp = P("hs", 2)
        thp = P("th", 2)
        rp = P("r", 1)
        smp = P("sm", 2)
        tmpp = P("tmp", 2)
        fin = P("fin", 2)

        # ---- weights / constants ----
        wc_sb = {}
        nk_sb = 3 if USE_DR else NK
        for d in range(2):
            for k in range(nk_sb):
                wt = wpool.tile([128, 1200], mdt, tag=f"wc_{d}_{k}")
                nc.sync.dma_start(out=wt, in_=wc_d.ap()[d, k])
                wc_sb[(d, k)] = wt
        wdr_sb = {}
        wdrh_sb = {}
        wdrxt_sb = None
        if USE_DR:
            for d in range(2):
                wt = wpool.tile([128, 2, WDRW], f8, tag=f"wdr_{d}")
                nc.sync.dma_start(out=wt, in_=wdr_d.ap()[d])
                wdr_sb[d] = wt
                wth = wpool.tile([128, 2, WDRW], f8, tag=f"wdrh_{d}")
                nc.sync.dma_start(out=wth, in_=wdrh_d.ap()[d])
                wdrh_sb[d] = wth
            wdrxt_sb = wpool.tile([128, 2, 448], f8, tag="wdrxt")
            nc.sync.dma_start(out=wdrxt_sb, in_=wdrxt_d.ap())
        conv_sb = wpool.tile([128, 3], mdt, tag="conv")
        nc.sync.dma_start(out=conv_sb, in_=conv_d.ap())
        fcw_sb = wpool.tile([128, 3 * NCLS], mdt, tag="fcw")
        nc.sync.dma_start(out=fcw_sb, in_=fcw_d.ap())
        fcb_sb = wpool.tile([1, NCLS], mdt, tag="fcb")
        nc.sync.dma_start(out=fcb_sb, in_=fcb_d.ap())
        ones_sb = wpool.tile([1, 128], mdt, tag="ones")
        nc.vector.memset(ones_sb, 1.0)
        sel44_sb = wpool.tile([128, 1], f32, tag="sel44")
        nc.sync.dma_start(out=sel44_sb, in_=sel44_d.ap())

        # ---- persistent state ----
        # h k-tiles, parity double-buffered: step t reads kt[t%2][d],
        # writes h_t into kt[(t+1)%2][d].
        kt = {}
        kt_dr = {}
        kt16 = {}
        for par in range(NPAR):
            for d in range(2):
                # j0|j1 h tiles are planes of ONE contiguous fp16 tile so
                # a single 1024-col DVE copy refreshes the whole fp8 DR
                # rhs per direction (the j1-plane write was already the
                # binding dependency, so the merge adds no stall)
                h16 = hp.tile([128, 2, BS], mdt, tag=f"kth_{par}_{d}")
                nc.vector.memset(h16, 0.0)
                kt16[(par, d)] = h16
                mixt = hp.tile([128, BS], mdt, tag=f"kt_{par}_{d}_2")
                nc.vector.memset(mixt, 0.0)
                kt[(par, d)] = [h16[:, 0, :], h16[:, 1, :], mixt]
                nc.sync.dma_start(out=kt[(par, d)][2][BIASROW:BIASROW + 1],
                                  in_=ones_d.ap())
                if USE_DR:
                    td = hp.tile([128, 2, BS], f8, tag=f"ktdr_{par}_{d}")
                    nc.vector.memset(td, 0.0)
                    kt_dr[(par, d)] = td
        # gate/cell state [tanh_g | c]: per (d, j) for the full 128-row
        # groups; ONE shared tile for the packed tails (d0 at partitions
        # 0:44, d1 at 64:108 — matmul col-group alignment needs base 64).
        gc = {}
        for d in range(2):
            for j in range(2):
                g = gcp.tile([128, 1024], mdt, tag=f"gc_{d}_{j}")
                nc.vector.memset(g, 0.0)
                gc[(d, j)] = g
        gc_t = gcp.tile([128, 1024], mdt, tag="gc_t")
        nc.vector.memset(gc_t, 0.0)
        r = []
        for j in range(3):
            rt = rp.tile([128, BS], f32, tag=f"r_{j}")
            nc.vector.memset(rt, 0.0)
            r.append(rt)
        # ssum is accumulated FOR FREE in r[2][44]: the mix k-tiles' bias
        # rows make hs[44] = 2 every step, so r[2][44] = 2 * sum_t e_t.
        # Zero the junk partitions of the two rotating hs buffers once so
        # the packed 1536-col tanh never sees NaN bit patterns.
        # hs rows 45:128 of the j2 segment are junk fed through the packed
        # tanh; nothing downstream reads those partitions of th (score and
        # accum slice [0:45]), so no zeroing is needed.

        def w_slice(d, k, col0, msz):
            return wc_sb[(d, k)][:, col0:col0 + msz]

        def attn_tanh(hs):
            # hs: [128, 1536] packed hsum (j0|j1|j2-tail) from the PREVIOUS
            # step.  One activation covers all three segments.
            th = thp.tile([128, 3 * BS], mdt, tag="th")
            nc.scalar.activation(out=th, in_=hs, func=AF.Tanh)
            return th

        def attn_tanh_seg(hs, th, k):
            # split form: one 512-col segment, interleavable between groups
            # so the big tanh never delays gate activations in the ACT FIFO
            if th is None:
                th = thp.tile([128, 3 * BS], mdt, tag="th")
            nc.scalar.activation(out=th[:, k * BS:k * BS + BS],
                                 in_=hs[:, k * BS:k * BS + BS], func=AF.Tanh)
            return th

        def attn_score(th, at_t):
            # conv weights are negated host-side: the matmul computes -a,
            # so e = exp(a) = (1 - sigmoid(-a)) / sigmoid(-a)
            #              = 1/sigmoid(-a) - 1  — two DVE ops, no multiply.
            a_ps = (at_t if ZLAYOUT == "banks" else alloc_att())[0:1]
            for k in range(3):
                pmax = 45 if k == 2 else 128
                nc.tensor.matmul(a_ps, lhsT=conv_sb[0:pmax, k:k + 1],
                                 rhs=th[0:pmax, k * BS:k * BS + BS],
                                 start=(k == 0), stop=(k == 2))
            sg = smp.tile([1, BS], f32, tag="sg")
            nc.scalar.activation(out=sg, in_=a_ps, func=AF.Sigmoid)
            om = smp.tile([1, BS], f32, tag="om")
            nc.vector.reciprocal(out=om, in_=sg)
            e16 = smp.tile([1, BS], mdt, tag="e16")
            nc.vector.tensor_scalar_add(out=e16, in0=om, scalar1=-1.0)
            return e16

        def attn_accum(hs, e16, at_t):
            eb_ps = at_t if ZLAYOUT == "banks" else alloc_att()
            nc.tensor.matmul(eb_ps, lhsT=ones_sb, rhs=e16, start=True, stop=True)
            # one 1x PSUM read to stage eb in SBUF fp16; the three products
            # then run all-SBUF all-fp16 (4x DVE mode) instead of 1x
            eb = tmpp.tile([128, BS], mdt, tag="eb")
            nc.vector.tensor_copy(out=eb, in_=eb_ps)
            for j in range(3):
                pmax = 45 if j == 2 else 128
                tmp = tmpp.tile([128, BS], mdt, tag=f"tmp{j}")
                nc.vector.tensor_mul(out=tmp[0:pmax],
                                     in0=hs[0:pmax, j * BS:j * BS + BS],
                                     in1=eb[0:pmax])
                radd = nc.gpsimd if RADD_GPSIMD else nc.vector
                radd.tensor_add(out=r[j][0:pmax], in0=r[j][0:pmax],
                                in1=tmp[0:pmax])

        def attn_front(th):
            """Score + exp + e-broadcast for the PREVIOUS step's hsum, run
            at the head of the step: both PSUM tiles drain within ~20% of
            the step, so they can share the 4-deep z-pair ring."""
            at1 = alloc_att()
            a_ps = at1[0:1]
            for k in range(3):
                pmax = 45 if k == 2 else 128
                nc.tensor.matmul(a_ps, lhsT=conv_sb[0:pmax, k:k + 1],
                                 rhs=th[0:pmax, k * BS:k * BS + BS],
                                 start=(k == 0), stop=(k == 2))
            sg = smp.tile([1, BS], f32, tag="sg")
            nc.scalar.activation(out=sg, in_=a_ps, func=AF.Sigmoid)
            om = smp.tile([1, BS], f32, tag="om")
            nc.vector.reciprocal(out=om, in_=sg)
            e16 = smp.tile([1, BS], mdt, tag="e16")
            nc.vector.tensor_scalar_add(out=e16, in0=om, scalar1=-1.0)
            eb_ps = alloc_att()
            nc.tensor.matmul(eb_ps[:, 0:BS], lhsT=ones_sb, rhs=e16,
                             start=True, stop=True)
            eb = tmpp.tile([128, BS], mdt, tag="eb")
            nc.vector.tensor_copy(out=eb, in_=eb_ps[:, 0:BS])
            return eb

        def attn_accum_sb(hs, eb):
            for j in range(3):
                pmax = 45 if j == 2 else 128
                tmp = tmpp.tile([128, BS], mdt, tag=f"tmp{j}")
                nc.vector.tensor_mul(out=tmp[0:pmax],
                                     in0=hs[0:pmax, j * BS:j * BS + BS],
                                     in1=eb[0:pmax])
                radd = nc.gpsimd if RADD_GPSIMD else nc.vector
                radd.tensor_add(out=r[j][0:pmax], in0=r[j][0:pmax],
                                in1=tmp[0:pmax])

        def attn_tail(hs):
            at_t = alloc_att()
            attn_accum(hs, attn_score(attn_tanh(hs), at_t), at_t)

        loop_cm = tc.For_i(0, repeat, 1) if repeat else None
        if loop_cm is not None:
            loop_cm.__enter__()

        pending_hs = None
        pending_th = None
        if not do_xdma:
            if USE_DR:
                xdr0 = xpool.tile([128, 2, BS], f8, tag="xdr")
                nc.vector.memset(xdr0, 0.0)
            else:
                xa0 = xpool.tile([128, BS], mdt, tag="xa")
                nc.vector.memset(xa0, 0.0)
                xb0 = xpool.tile([128, BS], mdt, tag="xb")
                nc.vector.memset(xb0, 0.0)

        # ---- time loop ----
        # Per-step group order: d0j0, d0j1, TAIL(packed, both dirs), d1j0,
        # d1j1.  The packed tail runs mid-step so its ACT/DVE chain (which
        # produces the h-tail rows every full group's mix pass needs next
        # step) completes ~2 groups before the step ends — the baseline's
        # tail-at-end ordering stalled PE ~1.4us at every step boundary.
        def issue_xdma(t):
            """DMA step t's x into tiles; mix x-tails go into the parity
            tile that step t will read."""
            tiles = {}
            if USE_DR:
                xdr = xpool.tile([128, 2, BS], f8, tag="xdr", name="xdr")
                nc.sync.dma_start(out=xdr, in_=xdr_d.ap()[t])
                tiles["xdr"] = xdr
            else:
                xa = xpool.tile([128, BS], mdt, tag="xa", name="xa")
                nc.sync.dma_start(out=xa, in_=xt_d.ap()[t, 0])
                xb = xpool.tile([128, BS], mdt, tag="xb", name="xb")
                nc.sync.dma_start(out=xb, in_=xt_d.ap()[t, 1])
                tiles["xa"], tiles["xb"] = xa, xb
            for d in range(2):
                nc.sync.dma_start(out=kt[(t % NPAR, d)][2][XTAIL:XTAIL + 44],
                                  in_=xt_d.ap()[t, 2][XTAIL:XTAIL + 44])
            return tiles

        # x for step 0 issued ahead of the loop; inside the loop each step
        # prefetches step t+1 so no PE pass ever waits on DGE latency
        pend_x = issue_xdma(0) if do_xdma else None

        for t in range(T):
            par, nxt = t % NPAR, (t + 1) % NPAR
            xa = xb = xdr = None
            if do_xdma:
                cur_x = pend_x
                if USE_DR:
                    xdr = cur_x["xdr"]
                else:
                    xa, xb = cur_x["xa"], cur_x["xb"]
                pend_x = issue_xdma(t + 1) if t + 1 < T else None
            elif USE_DR:
                xdr = xdr0
            else:
                xa, xb = xa0, xb0
            # previous step's attention tanh: emitted first — its input has
            # been ready since last step, so ACT starts immediately while
            # PE fills the first gate group.  TH_SPLIT staggers it across
            # the step instead (segment after each of the first 3 groups).
            pend = do_attn and pending_hs is not None
            pend_th = (attn_tanh(pending_hs)
                       if (pend and not TH_SPLIT) else None)
            pend_e16 = None

            hs = (hsp.tile([128, 3 * BS], mdt, tag="hs", name="hs")
                  if do_attn else None)

            def gate_matmuls(d, z4, moff, msz, plo):
                """Passes accumulating z for (direction d, row-tile at
                moff..moff+msz), output partitions plo..plo+msz."""
                sl = slice(plo, plo + msz)
                for gi in range(4):
                    zdst = z4[gi][sl, 0:ncols]
                    col0 = GOFF[gi] + moff
                    if USE_DR and plo == 0:
                        dc0 = 304 * gi + moff   # 16B-aligned gate starts
                        nc.tensor.matmul(
                            zdst, lhsT=wdr_sb[d][:, :, dc0:dc0 + msz],
                            rhs=xdr[:, :, 0:ncols],
                            start=True, stop=False, perf_mode=DRMODE)
                        nc.tensor.matmul(
                            zdst, lhsT=wdrh_sb[d][:, :, dc0:dc0 + msz],
                            rhs=kt_dr[(par, d)][:, :, 0:ncols],
                            start=False, stop=False, perf_mode=DRMODE)
                        nc.tensor.matmul(
                            zdst, lhsT=w_slice(d, 2, col0, msz),
                            rhs=kt[(par, d)][2][:, 0:ncols],
                            start=False, stop=True)
                    elif USE_DR:
                        # DoubleRow can't target a nonzero PSUM col-group:
                        # contract the same fp8 tiles half-by-half with
                        # plain passes (fp8 streams at bf16 rate)
                        dc0 = 304 * gi + moff
                        for s in range(2):
                            nc.tensor.matmul(
                                zdst, lhsT=wdr_sb[d][:, s, dc0:dc0 + msz],
                                rhs=xdr[:, s, 0:ncols],
                                start=(s == 0), stop=False)
                        for s in range(2):
                            nc.tensor.matmul(
                                zdst, lhsT=wdrh_sb[d][:, s, dc0:dc0 + msz],
                                rhs=kt_dr[(par, d)][:, s, 0:ncols],
                                start=False, stop=False)
                        nc.tensor.matmul(
                            zdst, lhsT=w_slice(d, 2, col0, msz),
                            rhs=kt[(par, d)][2][:, 0:ncols],
                            start=False, stop=True)
                    else:
                        rhsk = [(0, kt[(par, d)][0]), (1, kt[(par, d)][1]),
                                (2, kt[(par, d)][2]), (3, xa), (4, xb)]
                        for ki, (wk, rtile) in enumerate(rhsk):
                            nc.tensor.matmul(
                                zdst, lhsT=w_slice(d, wk, col0, msz),
                                rhs=rtile[:, 0:ncols],
                                start=(ki == 0), stop=(ki == len(rhsk) - 1))

            def full_group(d, j):
                moff, msz = MT[j]
                sl = slice(0, msz)
                swap = ZSWAP and d == 1 and (j == 1 or ZSWAP_D1J0)
                pfx = "zswap" if swap else "z"
                z4, zifp_ = alloc_z4(pfx)
                gate_matmuls(d, z4, moff, msz, 0)
                if not do_act:
                    return
                sif = sifp.tile([128, 1024], mdt, tag="sif")
                if zifp_ is not None:
                    nc.scalar.activation(out=sif[sl], in_=zifp_[sl],
                                         func=AF.Sigmoid)
                else:
                    nc.scalar.activation(out=sif[sl, 0:512], in_=z4[0][sl],
                                         func=AF.Sigmoid)
                    nc.scalar.activation(out=sif[sl, 512:1024], in_=z4[1][sl],
                                         func=AF.Sigmoid)
                gcj = gc[(d, j)]
                nc.scalar.activation(out=gcj[sl, 0:512], in_=z4[2][sl],
                                     func=AF.Tanh)
                so = sop.tile([128, BS], mdt, tag="so")
                nc.scalar.activation(out=so[sl], in_=z4[3][sl],
                                     func=AF.Sigmoid)
                if not do_dve:
                    return
                # c_new = sig_f * c + sig_i * tanh_g ; h = sig_o * tanh(c)
                p1 = p1p.tile([128, 1024], mdt, tag="p1")
                nc.vector.tensor_mul(out=p1[sl], in0=sif[sl], in1=gcj[sl])
                nc.vector.tensor_add(out=gcj[sl, 512:1024],
                                     in0=p1[sl, 0:512], in1=p1[sl, 512:1024])
                tcj = tcp.tile([128, BS], mdt, tag="tc")
                nc.scalar.activation(out=tcj[sl], in_=gcj[sl, 512:1024],
                                     func=AF.Tanh)
                # h_t lands directly in the next step's rhs k-tile
                if H_FP8_DIRECT and USE_DR:
                    # write h as fp8 straight into the DR rhs: removes the
                    # separate fp8 copy from the recurrence chain; the
                    # attention hsum reads the fp8 planes (small extra
                    # quantization on the attention path only)
                    if do_rec:
                        hdst = kt_dr[(nxt, d)][:, j, :]
                    else:
                        hdst = tcp.tile([128, 2, BS], f8, tag="hscr8")[:, 0, :]
                    nc.vector.tensor_mul(out=hdst[sl], in0=so[sl], in1=tcj[sl])
                    if d == 1 and do_attn:
                        nc.vector.tensor_add(out=hs[:, j * BS:j * BS + BS],
                                             in0=kt_dr[(nxt, 0)][:, j, :],
                                             in1=kt_dr[(nxt, 1)][:, j, :])
                    return
                if do_rec:
                    hdst = kt[(nxt, d)][j]
                else:  # timing probe: same traffic, no recurrence dep
                    hdst = tcp.tile([128, BS], mdt, tag="hscr")
                nc.vector.tensor_mul(out=hdst[sl], in0=so[sl], in1=tcj[sl])
                if USE_DR and do_rec and (j == 1 or not F8MERGE):
                    # fp8 copy feeds next step's DR h matmul; the fp16
                    # original stays for the attention hsum path.  With
                    # F8MERGE both planes go in one 1024-col copy after
                    # the j1 write.
                    eng = nc.gpsimd if F8COPY_GPSIMD else nc.vector
                    if F8MERGE:
                        eng.tensor_copy(out=kt_dr[(nxt, d)],
                                        in_=kt16[(nxt, d)])
                    else:
                        eng.tensor_copy(out=kt_dr[(nxt, d)][:, j, :],
                                        in_=hdst)
                if d == 1 and do_attn:
                    nc.vector.tensor_add(out=hs[:, j * BS:j * BS + BS],
                                         in0=kt[(nxt, 0)][j],
                                         in1=kt[(nxt, 1)][j])

            def tail_group():
                """Both directions' 44-row gate tails in one PSUM pair:
                d0 at partitions 0:44, d1 at 64:108 (col-group aligned).
                One ACT/DVE chain covers both directions — activation and
                vector cost scale with free-dim columns only, so packing
                partitions halves the tails' elementwise cost."""
                z4, zifp_ = alloc_z4("zt")
                if USE_DR:
                    # 6 passes per gate, one accumulation group per bank:
                    # x for BOTH dirs in one DoubleRow pass (xdr is shared;
                    # the combined lhsT has d0 tail cols at 0:44, d1 at
                    # 64:108 — DR is legal at col-group 0), then h per dir
                    # (d0 as DR at base 0; d1 as two plain fp8 half passes
                    # since DR can't target col-group 64), then the two
                    # fp16 mix passes.
                    for gi in range(4):
                        zt = z4[gi]
                        dc0 = 304 * gi + 256
                        col0 = GOFF[gi] + 256
                        nc.tensor.matmul(
                            zt[0:108, 0:ncols],
                            lhsT=wdrxt_sb[:, :, 112 * gi:112 * gi + 108],
                            rhs=xdr[:, :, 0:ncols],
                            start=True, stop=False, perf_mode=DRMODE)
                        nc.tensor.matmul(
                            zt[0:44, 0:ncols],
                            lhsT=wdrh_sb[0][:, :, dc0:dc0 + 44],
                            rhs=kt_dr[(par, 0)][:, :, 0:ncols],
                            start=False, stop=False, perf_mode=DRMODE)
                        for s in range(2):
                            nc.tensor.matmul(
                                zt[64:108, 0:ncols],
                                lhsT=wdrh_sb[1][:, s, dc0:dc0 + 44],
                                rhs=kt_dr[(par, 1)][:, s, 0:ncols],
                                start=False, stop=False)
                        nc.tensor.matmul(
                            zt[0:44, 0:ncols],
                            lhsT=w_slice(0, 2, col0, 44),
                            rhs=kt[(par, 0)][2][:, 0:ncols],
                            start=False, stop=False)
                        nc.tensor.matmul(
                            zt[64:108, 0:ncols],
                            lhsT=w_slice(1, 2, col0, 44),
                            rhs=kt[(par, 1)][2][:, 0:ncols],
                            start=False, stop=True)
                else:
                    for d in range(2):
                        gate_matmuls(d, z4, 256, 44, 0 if d == 0 else 64)
                if not do_act:
                    return
                sl = slice(0, 108)
                sif = sifp.tile([128, 1024], mdt, tag="sif")
                if zifp_ is not None:
                    nc.scalar.activation(out=sif[sl], in_=zifp_[sl],
                                         func=AF.Sigmoid)
                else:
                    nc.scalar.activation(out=sif[sl, 0:512], in_=z4[0][sl],
                                         func=AF.Sigmoid)
                    nc.scalar.activation(out=sif[sl, 512:1024], in_=z4[1][sl],
                                         func=AF.Sigmoid)
                nc.scalar.activation(out=gc_t[sl, 0:512], in_=z4[2][sl],
                                     func=AF.Tanh)
                so = sop.tile([128, BS], mdt, tag="so")
                nc.scalar.activation(out=so[sl], in_=z4[3][sl],
                                     func=AF.Sigmoid)
                if not do_dve:
                    return
                p1 = p1p.tile([128, 1024], mdt, tag="p1")
                nc.vector.tensor_mul(out=p1[sl], in0=sif[sl], in1=gc_t[sl])
                nc.vector.tensor_add(out=gc_t[sl, 512:1024],
                                     in0=p1[sl, 0:512], in1=p1[sl, 512:1024])
                tcj = tcp.tile([128, BS], mdt, tag="tc")
                nc.scalar.activation(out=tcj[sl], in_=gc_t[sl, 512:1024],
                                     func=AF.Tanh)
                if do_rec:
                    hd0, hd1 = kt[(nxt, 0)][2], kt[(nxt, 1)][2]
                else:
                    hd0 = tcp.tile([128, BS], mdt, tag="hscr")
                    hd1 = hd0
                nc.vector.tensor_mul(out=hd0[0:44], in0=so[0:44],
                                     in1=tcj[0:44])
                # d1: inputs at partitions 64:108, output realigned to 0:44
                # (DVE allows a shifted output when both inputs align).
                nc.vector.tensor_mul(out=hd1[0:44], in0=so[64:108],
                                     in1=tcj[64:108])
                if do_attn:
                    # rows 0:45 include the bias row (=1 in both mix tiles),
                    # so hs[44] = 2 and r[2][44] accumulates 2*sum(e) — the
                    # softmax denominator comes along for free.
                    nc.vector.tensor_add(out=hs[0:45, 2 * BS:3 * BS],
                                         in0=kt[(nxt, 0)][2][0:45],
                                         in1=kt[(nxt, 1)][2][0:45])

            if ATT_HEAD and do_attn:
                # th for THIS step's hsum is computed here (right after
                # each segment lands); score/exp/e-broadcast for the
                # PREVIOUS step run at the head so their z-ring tiles
                # drain early; r-accum stays at the end.
                th_cur = None
                full_group(0, 0)
                if pend:
                    pend_eb = attn_front(pending_th)
                full_group(0, 1)
                tail_group()
                th_cur = attn_tanh_seg(hs, th_cur, 2)
                full_group(1, 0)
                th_cur = attn_tanh_seg(hs, th_cur, 0)
                full_group(1, 1)
                th_cur = attn_tanh_seg(hs, th_cur, 1)
                if pend:
                    attn_accum_sb(pending_hs, pend_eb)
                pending_th = th_cur
                pending_hs = hs
                continue
            full_group(0, 0)
            if pend and TH_SPLIT:
                pend_th = attn_tanh_seg(pending_hs, None, 0)
            full_group(0, 1)
            if pend and TH_SPLIT:
                attn_tanh_seg(pending_hs, pend_th, 1)
            if pend and TH_SPLIT and SCORE_EARLY:
                attn_tanh_seg(pending_hs, pend_th, 2)
            tail_group()
            if pend and TH_SPLIT and not SCORE_EARLY:
                attn_tanh_seg(pending_hs, pend_th, 2)
            if pend and SCORE_EARLY:
                at_t = alloc_att()
                if TH_SPLIT:
                    pend_e16 = attn_score(pend_th, at_t)
            full_group(1, 0)
            if pend and not SCORE_EARLY:
                at_t = alloc_att()
                if TH_SPLIT:
                    pend_e16 = attn_score(pend_th, at_t)
            if pend and ACC_MID and TH_SPLIT:
                attn_accum(pending_hs, pend_e16, at_t)
            full_group(1, 1)
            # attention accumulation LAST: its DVE ops sit behind every
            # h-chain op of this step in the DVE FIFO, so they fill the
            # step boundary instead of delaying the recurrence.
            if pend and not (ACC_MID and TH_SPLIT):
                if not TH_SPLIT:
                    pend_e16 = attn_score(pend_th, at_t)
                attn_accum(pending_hs, pend_e16, at_t)
            pending_hs = hs

        if do_attn:
            if ATT_HEAD:
                attn_accum_sb(pending_hs, attn_front(pending_th))
            else:
                attn_tail(pending_hs)

        if loop_cm is not None:
            loop_cm.__exit__(None, None, None)

        # ---- tail: hStar = tanh(r / s); logits; softmax ----
        # softmax denominator: r[2][44] = 2 * sum_t e_t (bias-row trick);
        # ACT moves it from partition 44 to partition 0, the *2 is folded
        # into the rs16 copy's scale.
        rs = smp.tile([1, BS], f32, tag="rs")
        if do_attn:
            # partition 44 -> 0 via a one-hot selector matmul (compute
            # engines can't start an access at partition 44)
            srow_ps = zp.tile([128, BS], f32, tag="z", name="srow_ps")[0:1]
            nc.tensor.matmul(srow_ps, lhsT=sel44_sb[0:45, 0:1],
                             rhs=r[2][0:45], start=True, stop=True)
            nc.vector.reciprocal(out=rs, in_=srow_ps)
        else:
            srow = smp.tile([1, BS], f32, tag="srow")
            nc.vector.memset(srow, 1.0)   # timing probes: keep 1/s finite
            nc.vector.reciprocal(out=rs, in_=srow)
        rs16 = smp.tile([1, BS], mdt, tag="rs16")
        nc.scalar.activation(out=rs16, in_=rs, func=AF.Copy, scale=2.0)
        def _att_ps(shape):
            zt = zp.tile([128, BS], f32, tag="z", name="attps")
            return zt[0:shape[0], 0:shape[1]]
        rsb = _att_ps([128, BS])
        nc.tensor.matmul(rsb, lhsT=ones_sb, rhs=rs16, start=True, stop=True)
        hst = []
        for j in range(3):
            hn = fin.tile([128, BS], f32, tag=f"hn{j}")
            nc.vector.tensor_mul(out=hn, in0=r[j], in1=rsb)
            hj = fin.tile([128, BS], mdt, tag=f"hst{j}")
            nc.scalar.activation(out=hj, in_=hn, func=AF.Tanh)
            hst.append(hj)
        for bt in range(BS // 128):
            fcp = _att_ps([128, NCLS])
            for j in range(3):
                nc.tensor.matmul(fcp, lhsT=hst[j][:, bt * 128:(bt + 1) * 128],
                                 rhs=fcw_sb[:, j * NCLS:(j + 1) * NCLS],
                                 start=(j == 0), stop=False)
            nc.tensor.matmul(fcp, lhsT=ones_sb, rhs=fcb_sb, start=False, stop=True)
            mx = fin.tile([128, 1], f32, tag="mx")
            nc.vector.reduce_max(out=mx, in_=fcp, axis=AX.X)
            nmx = fin.tile([128, 1], f32, tag="nmx")
            nc.vector.tensor_scalar_mul(out=nmx, in0=mx, scalar1=-1.0)
            ex = fin.tile([128, NCLS], f32, tag="ex")
            nc.scalar.activation(out=ex, in_=fcp, func=AF.Exp, bias=nmx)
            sm = fin.tile([128, 1], f32, tag="smm")
            nc.vector.reduce_sum(out=sm, in_=ex, axis=AX.X)
            nc.vector.reciprocal(out=sm, in_=sm)
            ot = fin.tile([128, NCLS], f32, tag="ot")
            nc.vector.tensor_scalar_mul(out=ot, in0=ex, scalar1=sm)
            nc.sync.dma_start(out=out_d.ap()[bt * 128:(bt + 1) * 128], in_=ot)

    return nc


def _prep(x, w_ih, w_hh, b_ih, b_hh, conv_w, fc_w, fc_b, np_mdt):
    """Host-side layout prep (shared across cores + per-core x shards).

    Merged contraction rows (640 = 5 k-tiles of 128):
      tile 0: h[0:128]        tile 1: h[128:256]
      tile 2: h[256:300] at parts 0..43, bias (const-1 row) at part 44,
              x[256:300] at parts 64..107, zeros elsewhere
      tile 3: x[0:128]        tile 4: x[128:256]
    """
    bias = (b_ih + b_hh).astype(np.float32)  # [2, 1200]
    wc = np.zeros((2, NK, 128, 1200), np.float32)
    for d in range(2):
        comb = np.zeros((NK * 128, 1200), np.float32)
        comb[0:256] = w_hh[d].T[0:256]
        comb[256:300] = w_hh[d].T[256:300]
        comb[256 + BIASROW] = bias[d]
        comb[256 + XTAIL:256 + XTAIL + 44] = w_ih[d].T[256:300]
        comb[384:512] = w_ih[d].T[0:128]
        comb[512:640] = w_ih[d].T[128:256]
        wc[d] = comb.reshape(NK, 128, 1200)

    def h_pack(vec_or_mat, width):
        """Pack [300(, width)] h-feature data into the 3-tile h k-layout."""
        out = np.zeros((3, 128, width), np.float32)
        v = vec_or_mat.reshape(H, width)
        out[0] = v[0:128]
        out[1] = v[128:256]
        out[2, 0:44] = v[256:300]
        return out

    # conv NEGATED: the score matmul computes -a so the in-loop exp trick
    # is e = 1/sigmoid(-a) - 1 (two DVE ops)
    convp = np.ascontiguousarray(
        -h_pack(conv_w, 1).reshape(3, 128).T)         # [128, 3]
    fcw = np.ascontiguousarray(
        h_pack(fc_w.T, NCLS).transpose(1, 0, 2).reshape(128, 3 * NCLS))

    sel44 = np.zeros((128, 1), np.float32)
    sel44[BIASROW, 0] = 1.0
    shared = {
        "wc": wc.astype(np_mdt),
        "convp": convp.astype(np_mdt),
        "fcw": fcw.astype(np_mdt),
        "fcb": fc_b.reshape(1, NCLS).astype(np_mdt),
        "onesrow": np.ones((1, BS), np.float32).astype(np_mdt),
        "sel44": sel44,
    }
    if USE_DR:
        import ml_dtypes
        np_f8 = ml_dtypes.float8_e4m3
        # wdr[d, p, s, 304*g + r] = w_ih[d][300*g + r, 128*s + p]
        wdr = np.zeros((2, 128, 2, WDRW), np.float32)
        wdrh = np.zeros((2, 128, 2, WDRW), np.float32)
        for d in range(2):
            tmp = w_ih[d][:, 0:256].reshape(1200, 2, 128)
            tmph = w_hh[d][:, 0:256].reshape(1200, 2, 128)
            for g in range(4):
                wdr[d, :, :, 304 * g:304 * g + 300] = (
                    tmp[300 * g:300 * g + 300].transpose(2, 1, 0))
                wdrh[d, :, :, 304 * g:304 * g + 300] = (
                    tmph[300 * g:300 * g + 300].transpose(2, 1, 0))
        shared["wdr"] = wdr.astype(np_f8)
        shared["wdrh"] = wdrh.astype(np_f8)
        # combined x gate-tail block: per gate gi (112-col pitch, 16B
        # aligned for DR), cols 0:44 = d0 rows 256:300, 64:108 = d1
        wdrxt = np.zeros((128, 2, 448), np.float32)
        for g in range(4):
            for d in range(2):
                tmp = w_ih[d][:, 0:256].reshape(1200, 2, 128)
                wdrxt[:, :, 112 * g + 64 * d:112 * g + 64 * d + 44] = (
                    tmp[300 * g + 256:300 * g + 300].transpose(2, 1, 0))
        shared["wdrxt"] = wdrxt.astype(np_f8)

    # x: [B, H, T] -> per-core [T, 3, 128, BS]:
    # slot 0 = x[0:128], slot 1 = x[128:256],
    # slot 2 = zeros with x[256:300] at parts 64..107.
    xs = np.ascontiguousarray(np.transpose(x, (2, 1, 0)))  # [T, H, B]
    xp = np.zeros((T, 3, 128, B), np.float32)
    xp[:, 0] = xs[:, 0:128]
    xp[:, 1] = xs[:, 128:256]
    xp[:, 2, XTAIL:XTAIL + 44] = xs[:, 256:300]
    xp = xp.reshape(T, 3, 128, NCORES, BS)
    if USE_DR:
        # xdr[t, p, s, b] = x[t, 128*s + p, b]
        xdr = xs[:, 0:256].reshape(T, 2, 128, NCORES, BS).transpose(0, 2, 1, 3, 4)
    in_maps = []
    for c in range(NCORES):
        m = dict(shared)
        m["xt"] = np.ascontiguousarray(xp[:, :, :, c]).astype(np_mdt)
        if USE_DR:
            import ml_dtypes
            m["xdr"] = np.ascontiguousarray(xdr[:, :, :, c]).astype(
                ml_dtypes.float8_e4m3)
        in_maps.append(m)
    return in_maps


def _np_mdt(mdt_name):
    return np.float16 if mdt_name == "float16" else (
        __import__("ml_dtypes").bfloat16 if mdt_name == "bfloat16" else np.float32)


def _runner(repeat=0, variant="full"):
    key = (MM_DT_NAME, repeat, variant)
    if key not in _CACHE:
        _CACHE[key] = _Runner(_build(MM_DT_NAME, repeat=repeat,
                                     variant=variant), NCORES)
    return _CACHE[key]


def _in_maps(inputs_f32):
    return _prep(*inputs_f32, _np_mdt(MM_DT_NAME))


def _inputs_f32(x, w_ih, w_hh, b_ih, b_hh, conv_w, fc_w, fc_b):
    return [np.asarray(a, np.float32) for a in
            (x, w_ih, w_hh, b_ih, b_hh, conv_w, fc_w, fc_b)]


def kernel(x, w_ih, w_hh, b_ih, b_hh, conv_w, fc_w, fc_b):
    runner = _runner(repeat=0)
    in_maps = _in_maps(_inputs_f32(x, w_ih, w_hh, b_ih, b_hh,
                                   conv_w, fc_w, fc_b))
    results = runner.run(in_maps)
    out = np.concatenate([r["out"] for r in results], axis=0)
    return out.astype(np.float32)


def bench(x, w_ih, w_hh, b_ih, b_hh, conv_w, fc_w, fc_b, iters=5):
    runner = _runner(repeat=0)
    in_maps = _in_maps(_inputs_f32(x, w_ih, w_hh, b_ih, b_hh,
                                   conv_w, fc_w, fc_b))
    return runner.bench(in_maps, iters=iters)


def measure_exec_ns(inputs, r_lo=1, r_hi=301, iters=16):
    """Device execution time of one full forward pass, in ns.

    The axon tunnel adds a fixed ~70-80 ms completion-notification latency
    to every blocking call, independent of what the NEFF does (measured:
    a trivial 4-instruction kernel takes the same wall time as the full
    LSTM).  To measure hardware execution, both builds wrap the whole
    T-step forward in a hardware For_i loop (r_lo vs r_hi iterations,
    identical instruction stream per iteration); the slope
    (median_wall(r_hi) - median_wall(r_lo)) / (r_hi - r_lo) is the
    steady-state on-device time of one forward pass with the constant
    latency cancelled.  Samples are interleaved so network drift affects
    both arms equally; the first two samples per arm are discarded (the
    call right after the warm call consistently measures ~25 ms fast —
    a tunnel batching artifact that corrupted the old min() estimator
    asymmetrically) and the median kills remaining outliers.
    """
    import statistics
    import time
    in_maps = _in_maps(_inputs_f32(**inputs) if isinstance(inputs, dict)
                       else _inputs_f32(*inputs))
    runners = {rep: _runner(repeat=rep) for rep in (r_lo, r_hi)}
    dev_in = {rep: runners[rep].put_inputs(in_maps) for rep in (r_lo, r_hi)}
    for rep in (r_lo, r_hi):
        runners[rep].call(dev_in[rep])  # warm
    walls = {r_lo: [], r_hi: []}
    for i in range(iters):
        for rep in (r_lo, r_hi):
            t0 = time.perf_counter()
            runners[rep].call(dev_in[rep])
            walls[rep].append(time.perf_counter() - t0)
        if i == 1:
            for rep in (r_lo, r_hi):
                walls[rep].clear()
    lo = statistics.median(walls[r_lo])
    hi = statistics.median(walls[r_hi])
    ns = (hi - lo) * 1e9 / (r_hi - r_lo)
    return max(int(ns), 1), walls

